# revision 70
# baseline (speedup 1.0000x reference)
"""Trainium2 Bass kernel for nn_GAT_GraphSAGE (N=12000, E=192000, F=35, B=64).

Sharding: attention rows (softmax row i = K_new index) sharded 1500/core on
8 cores; one AllGather of post-attention h per 512-row piece (bf16); SAGE
sharded by dst with a batched dma_gather of h[src] rows + one-hot-matmul
scatter; per-core global-max-pool + MLP head on that core's 8 graphs.

v4 = v2 attention (3 i-pieces of 512, exp on ACT is the in-attention
bottleneck) + restructured SAGE scatter:
- Gather slots deduplicated per (src, piece, 512-col dst group) and packed
  into full 128-slot chunks (24448 -> 16384 slots; GpSimd SWDGE descriptor
  generation measures ~7ns/slot on HW, and is the serial bottleneck after
  attention ends).
- Each chunk scatters via a [64,512] one-hot matmul pair: top/bottom
  64-slot halves write opposite psum partition halves, alternating per
  chunk so all 4 PE quadrants stay busy; the two halves are summed into
  aggS once per (piece, group).
- Self-core edges keep the dense local block adjacency, restructured to
  512-col dst groups with the same quadrant packing.
- MLP head weights in bf16 (kills the fp32 LOW/HIGH double-pass matmuls
  in the tail).
"""
import math
import numpy as np
import ml_dtypes

BF16 = ml_dtypes.bfloat16

N, E, F, B = 12000, 192000, 35, 64
F1 = F + 1
NCORE = 8
ROWS = N // NCORE            # 1500
ICH = 512
NI = 3
IPAD = ICH * NI              # 1536
JT = 94                      # j chunks of 128
JPAD = JT * 128              # 12032
XW = 12064                   # padded x~^T width (covers 7*1500 + 1536)
DBLK = 12                    # dst blocks (128 each) per core
NGRP = 3                     # dst groups of 512 cols
GB = B // NCORE              # 8 graphs per core
HPAD = 128                   # h row padded to 128 bf16 (256B) for dma_gather
GRAPH_BOUNDS = [int(math.ceil(g * (N / B))) for g in range(GB + 1)]
# 6 AllGather pieces of 2 h-blocks each (256/256/256/256/256/220 local
# rows): each is triggered as soon as its two h blocks are written, so the
# last piece's AllGather lands ~45us earlier than a monolithic 476-row one.
NP = 6
PLO = [p * 256 for p in range(NP)]
PHI = [min((p + 1) * 256, ROWS) for p in range(NP)]
PLEN = [PHI[p] - PLO[p] for p in range(NP)]
NSPL = 2                     # gather calls per piece


# --------------------------------------------------------------------------
# host-side preprocessing
# --------------------------------------------------------------------------

def _prep_weights(p):
    f64 = np.float64
    f32 = np.float32
    Wq, bq = p['Wq'].astype(f64), p['bq'].astype(f64)
    Wk, bk = p['Wk'].astype(f64), p['bk'].astype(f64)
    Wv, bv = p['Wv'].astype(f64), p['bv'].astype(f64)
    W3c, b3 = p['W3'][:, :, 1].astype(f64), p['b3'].astype(f64)
    W5c, b5 = p['W5'][:, :, 2].astype(f64), p['b5'].astype(f64)
    Wl, bl = p['Wl'].astype(f64), p['bl'].astype(f64)
    Wl1, Wl2, Wl3 = Wl[:, :F], Wl[:, F:2 * F], Wl[:, 2 * F:]

    # K_new = x~ @ Wkn~  (F1 -> F affine, includes 1/sqrt(F))
    Weff = W3c.T @ Wl1.T + W5c.T @ Wl2.T + Wl3.T
    beff = b3 @ Wl1.T + b5 @ Wl2.T + bl
    Wkn = Wk.T @ Weff
    bkn = bk @ Weff + beff
    s = 1.0 / np.sqrt(F)
    Wkn_aug = np.vstack([Wkn, bkn[None, :]]) * s          # [F1, F]
    Wq_aug = np.vstack([Wq.T, bq[None, :]])               # [F1, F]
    M = Wkn_aug @ Wq_aug.T                                # [F1, F1]

    Wva = np.zeros((F1, F1))
    Wva[:F, :F] = Wv.T
    Wva[F, :F] = bv
    Wva[F, F] = 1.0                                       # denominator column
    out = {'M': M, 'Wva': Wva.astype(BF16)}
    out['WllT'] = np.ascontiguousarray(p['Wll'].T).astype(BF16)
    out['WlrT'] = np.ascontiguousarray(p['Wlr'].T).astype(BF16)
    out['bll'] = p['bll'].astype(f32).reshape(F, 1)
    out['Wg1T'] = np.ascontiguousarray(p['Wg1'].T).astype(f32)   # [35,1500]
    bg1 = np.zeros((128, 12), f32)
    bg1.T.reshape(-1)[:1500] = p['bg1'].astype(f32)
    out['bg1'] = bg1
    w2 = np.zeros((12 * 128, 128), f32)
    w2[:1500, :] = p['Wg2'].T.astype(f32)
    out['Wg2Tr'] = np.ascontiguousarray(
        w2.reshape(12, 128, 128).transpose(1, 0, 2).reshape(128, 12 * 128))
    out['bg2'] = p['bg2'].astype(f32).reshape(128, 1)
    out['WoT'] = p['Wo'].astype(f32).reshape(1, 128).T.copy()
    out['bo'] = float(np.asarray(p['bo']).reshape(-1)[0])
    return out


def _prep_x(x, M):
    """Host: x~^T (bf16), x~ natural chunked (bf16), per-core KQT."""
    x64 = np.asarray(x, np.float64)
    xa = np.concatenate([x64, np.ones((N, 1))], axis=1)       # [N, F1]
    xaT = np.zeros((F1, XW))
    xaT[:, :N] = xa.T                                         # pad cols zero
    xh = xaT.astype(BF16)                                     # [F1, XW]

    # natural chunks for the U accumulation: [128, JT, F1]
    xn = np.zeros((128, JT, F1))
    flat = xaT[:, :JPAD].T                                    # [JPAD, F1]
    xn[:, :, :] = flat.reshape(JT, 128, F1).transpose(1, 0, 2)
    xn = np.ascontiguousarray(xn.reshape(128, JT * F1)).astype(BF16)

    KQ = []
    for c in range(NCORE):
        sl = xaT[:, c * ROWS: c * ROWS + IPAD]                # [F1, IPAD]
        KQ.append(np.ascontiguousarray(M.T @ sl).astype(BF16))
    return xh, xn, KQ


def _prep_vl(x, p):
    """Per-core natural V' local [128, DBLK*F] f32 (for the residual)."""
    f64 = np.float64
    Wv, bv = p['Wv'].astype(f64), p['bv'].astype(f64)
    x64 = np.asarray(x, np.float64)
    V = x64 @ Wv.T + bv                                       # [N, F]
    out = []
    for c in range(NCORE):
        vl = np.zeros((DBLK * 128, F))
        vl[:ROWS] = V[c * ROWS:(c + 1) * ROWS]
        out.append(np.ascontiguousarray(
            vl.reshape(DBLK, 128, F).transpose(1, 0, 2).reshape(128, DBLK * F)
        ).astype(np.float32))
    return out


def _prep_edges(edge_index):
    """Non-self edges keyed (piece p of src, dst group g of 512 cols),
    DEDUPED per (src, p, g): each gathered slot is a unique src row whose
    P columns cover every dst it feeds in that group.  Chunk stream is
    p-major then g-major with S_pg (global max over cores) 128-slot chunks
    per (p, g).  Self-core edges go to a dense local block adjacency Aloc
    laid out per (group, src block).
    """
    src = np.asarray(edge_index[0], np.int64)
    dst = np.asarray(edge_index[1], np.int64)
    deg = np.bincount(dst, minlength=N).astype(np.float64)
    recip = (1.0 / np.maximum(deg, 1.0)).astype(np.float32)

    core_of = dst // ROWS
    dloc = dst - core_of * ROWS
    grp_of = dloc // 512
    sc = src // ROWS
    sr = src - sc * ROWS
    piece_of = np.minimum(sr // 256, NP - 1)
    selfm = sc == core_of
    ns = ~selfm

    # ---- self edges: dense [128, NGRP * DBLK * 512] block adjacency ----
    Als = []
    for c in range(NCORE):
        m = selfm & (core_of == c)
        A = np.zeros((128, NGRP * DBLK * 512), np.float32)
        ssb = sr[m] // 128              # src block
        ssp = sr[m] - ssb * 128         # src pos in block
        g = grp_of[m]
        rel = dloc[m] - g * 512
        np.add.at(A, (ssp, (g * DBLK + ssb) * 512 + rel), 1.0)
        Als.append(np.ascontiguousarray(A.astype(BF16)))

    # ---- non-self: unique (core, piece, grp, src) slots ----
    key = ((core_of * NP + piece_of) * NGRP + grp_of) * N + src
    ukey, inv = np.unique(key[ns], return_inverse=True)
    ucpg = ukey // N
    usrc = ukey - ucpg * N
    uc = ucpg // (NP * NGRP)
    up = (ucpg // NGRP) % NP
    ug = ucpg % NGRP
    counts = np.zeros((NCORE, NP, NGRP), np.int64)
    np.add.at(counts, (uc, up, ug), 1)
    S_pg = np.ceil(counts.max(axis=0) / 128).astype(np.int64)  # [NP, NGRP]
    CH = int(S_pg.sum())

    ch_off = np.zeros((NP, NGRP), np.int64)
    acc = 0
    for p in range(NP):
        for g in range(NGRP):
            ch_off[p, g] = acc
            acc += S_pg[p, g]

    # slot position within its (c,p,g) cell (ukey sorted -> contiguous)
    cell_id = ucpg
    cell_starts = np.searchsorted(cell_id, np.arange(NCORE * NP * NGRP))
    slot_in_cell = np.arange(len(ukey)) - cell_starts[cell_id]
    gslot = ch_off[up, ug] * 128 + slot_in_cell

    # gather position within h_full_p
    plen_arr = np.array(PLEN)[up]
    plo_arr = np.array(PLO)[up]
    su_c = usrc // ROWS
    su_r = usrc - su_c * ROWS
    pos = su_c * plen_arr + (su_r - plo_arr)

    gidx, Ps = [], []
    for c in range(NCORE):
        mu = uc == c
        idx_c = np.zeros(CH * 128, np.int16)
        idx_c[gslot[mu]] = pos[mu].astype(np.int16)
        gidx.append(np.ascontiguousarray(
            np.tile(idx_c.reshape(-1, 16).T, (8, 1))))
        Ps.append(np.zeros((128, CH * 512), np.float32))

    # fill P: each non-self edge scatters its unique slot to its rel col
    e_slot = gslot[inv]
    e_core = core_of[ns]
    e_rel = dloc[ns] - grp_of[ns] * 512
    e_chunk = e_slot // 128
    e_sp = e_slot - e_chunk * 128
    for c in range(NCORE):
        m = e_core == c
        np.add.at(Ps[c], (e_sp[m], e_chunk[m] * 512 + e_rel[m]), 1.0)
    Ps = [np.ascontiguousarray(P.astype(BF16)) for P in Ps]

    recipT = []
    for c in range(NCORE):
        r = np.ones(IPAD, np.float32)
        r[:ROWS] = recip[c * ROWS:(c + 1) * ROWS]
        recipT.append(np.ascontiguousarray(np.broadcast_to(r, (F, IPAD))))
    return gidx, Ps, Als, recipT, tuple(int(v) for v in S_pg.reshape(-1))


# --------------------------------------------------------------------------
# device program
# --------------------------------------------------------------------------

def _emit_body(nc, tc, d, S_pg, bo_const):
    import concourse.tile as tile
    import os
    from concourse import mybir
    from concourse.tile import add_dep_helper

    BIS = int(os.environ.get('KBISECT', '0'))
    f32 = mybir.dt.float32
    bf16 = mybir.dt.bfloat16
    S_pg = [list(S_pg[p * NGRP:(p + 1) * NGRP]) for p in range(NP)]
    NCH = [int(sum(S_pg[p])) for p in range(NP)]       # chunks per piece
    CH = sum(NCH)
    POFF = [int(sum(NCH[:p])) for p in range(NP)]      # piece chunk offsets

    with tc.tile_pool(name="const", bufs=1) as constp, \
         tc.tile_pool(name="main", bufs=1) as main, \
         tc.tile_pool(name="gat", bufs=12) as gat, \
         tc.tile_pool(name="pin", bufs=3) as pin, \
         tc.tile_pool(name="sin", bufs=1) as sin:
        # ---- inputs (leading xhT slices + xn first so exp starts early) ----
        KQT = main.tile([128, IPAD], bf16, name="KQT")
        nc.sync.dma_start(out=KQT[0:F1, :], in_=d['KQ'][:, :])
        nc.sync.dma_start(out=KQT[64:64 + F1, :], in_=d['KQ'][:, :])
        xhT = main.tile([128, XW], bf16, name="xhT")
        xn = main.tile([128, JT * F1], bf16, name="xn")
        HW = XW // 8
        XNW = (JT * F1) // 4
        for q in range(8):
            nc.sync.dma_start(out=xhT[0:F1, q * HW:(q + 1) * HW],
                              in_=d['xh'][:, q * HW:(q + 1) * HW])
            nc.sync.dma_start(out=xhT[64:64 + F1, q * HW:(q + 1) * HW],
                              in_=d['xh'][:, q * HW:(q + 1) * HW])
            if q < 4:
                nc.sync.dma_start(out=xn[:, q * XNW:(q + 1) * XNW],
                                  in_=d['xn'][:, q * XNW:(q + 1) * XNW])
        Wva_t = constp.tile([F1, F1], bf16, name="Wva_t")
        nc.sync.dma_start(out=Wva_t[:], in_=d['Wva'][:, :])
        Vl = main.tile([128, DBLK * F], f32, name="Vl")
        nc.sync.dma_start(out=Vl[:], in_=d['Vl'][:, :])
        ident_t = constp.tile([128, 128], bf16, name="ident_t")
        nc.sync.dma_start(out=ident_t[:], in_=d['ident'][:, :])

        hnat = main.tile([128, DBLK, HPAD], bf16, name="hnat")
        nc.vector.memset(hnat[:, :, F:HPAD], 0.0)
        aggS = main.tile([F, IPAD], f32, name="aggS")
        nc.vector.memset(aggS[:], 0.0)

        # ---------------- attention ----------------
        # groups of 2 j-chunks; one [128,1024] exp per group (2 PSUM banks,
        # double-buffered). U' = sum_j x~_j^T exp[j,:] accumulated in two
        # K-half chains (row groups 0/64 -> UC/UD); V-projection after.
        # UC/UD double-buffered (Up bufs=2) so piece ci+1's U-chain starts
        # without waiting for ci's h post-processing.
        GROUPS = [(g * 2, min(2, JT - g * 2)) for g in range((JT + 1) // 2)]
        exp_f = mybir.ActivationFunctionType.Exp
        with tc.tile_pool(name="mm1p", bufs=2, space="PSUM") as mm1p, \
             tc.tile_pool(name="Up", bufs=2, space="PSUM") as Upp, \
             tc.tile_pool(name="esb", bufs=3) as esb, \
             tc.tile_pool(name="usb", bufs=2) as usb, \
             tc.tile_pool(name="hsm", bufs=4) as hsmall:
            for ci in range(NI):
                UC = Upp.tile([128, ICH], f32, tag="uc", name="UC")
                UD = Upp.tile([128, ICH], f32, tag="ud", name="UD")
                prev = None
                for (j0, glen) in GROUPS:
                    ps = mm1p.tile([128, 2 * ICH], f32, space="PSUM",
                                   tag="s", name="pss")
                    for k in range(glen):
                        j = j0 + k
                        r = 64 * (j & 1)
                        for ch in range(2):
                            nc.tensor.matmul(
                                out=ps[64 * ch:64 * ch + 64,
                                       k * ICH:(k + 1) * ICH],
                                lhsT=xhT[r:r + F1,
                                         j * 128 + 64 * ch:
                                         j * 128 + 64 * ch + 64],
                                rhs=KQT[r:r + F1,
                                        ci * ICH:(ci + 1) * ICH],
                                start=True, stop=True)
                    et = esb.tile([128, 3 * ICH], bf16, tag="e", name="et")
                    nc.scalar.activation(out=et[:, :glen * ICH],
                                         in_=ps[:, :glen * ICH], func=exp_f)
                    if prev is not None:
                        pe, pj0, pglen = prev
                        for k in range(pglen):
                            j = pj0 + k
                            for r in range(2):
                                nc.tensor.matmul(
                                    out=(UC if r == 0 else UD)[0:F1, :],
                                    lhsT=xn[64 * r:64 * r + 64,
                                            j * F1:(j + 1) * F1],
                                    rhs=pe[64 * r:64 * r + 64,
                                           k * ICH:(k + 1) * ICH],
                                    start=(j == 0), stop=False,
                                    skip_group_check=True)
                    prev = (et, j0, glen)
                pe, pj0, pglen = prev
                for k in range(pglen):
                    j = pj0 + k
                    for r in range(2):
                        nc.tensor.matmul(
                            out=(UC if r == 0 else UD)[0:F1, :],
                            lhsT=xn[64 * r:64 * r + 64, j * F1:(j + 1) * F1],
                            rhs=pe[64 * r:64 * r + 64,
                                   k * ICH:(k + 1) * ICH],
                            start=False, stop=(k == pglen - 1),
                            skip_group_check=True)
                # combine K-halves -> U'sb bf16 [F1, 512]
                # (avoid a two-PSUM-operand tensor_tensor: copy then add)
                Ucs = usb.tile([F1, ICH], f32, tag="ucs", name="Ucs")
                nc.vector.tensor_copy(out=Ucs[:], in_=UC[0:F1, :])
                Usb = usb.tile([F1, ICH], bf16, tag="usb", name="Usb")
                nc.vector.tensor_add(out=Usb[:], in0=Ucs[:],
                                     in1=UD[0:F1, :])
                # h natural: hraw[i,g] = sum_f U'sb[f,i] Wva[f,g]
                # (two 64-col halves to stay in the 64x64 tile grid)
                for t in range(4):
                    blk = ci * 4 + t
                    for ch in range(2):
                        last_att_mm = nc.tensor.matmul(
                            out=UD[64 * ch:64 * ch + 64,
                                   t * 128:t * 128 + F1],
                            lhsT=Usb[:, t * 128 + 64 * ch:
                                     t * 128 + 64 * ch + 64],
                            rhs=Wva_t[:],
                            start=True, stop=True, skip_group_check=True)
                    hraw = UD[:, t * 128:t * 128 + F1]
                    rec = hsmall.tile([128, 1], f32, tag="rec", name="rec")
                    nc.vector.reciprocal(out=rec[:], in_=hraw[:, F:F1])
                    hh = hsmall.tile([128, F], f32, tag="hh", name="hh")
                    nc.vector.scalar_tensor_tensor(
                        out=hh[:], in0=hraw[:, :F], scalar=rec[:],
                        in1=Vl[:, blk * F:(blk + 1) * F],
                        op0=mybir.AluOpType.mult,
                        op1=mybir.AluOpType.add)
                    nc.vector.tensor_scalar_max(out=hnat[:, blk, :F],
                                                in0=hh[:], scalar1=0.0)
                    lo = blk * 128
                    nrows = min(128, max(0, ROWS - lo))
                    if nrows > 0:
                        nc.sync.dma_start(
                            out=d['h_loc'][lo:lo + nrows, :],
                            in_=hnat[:nrows, blk, :])
                    if t % 2 == 1:
                        # AG half-piece as soon as its 2 blocks are out
                        # (collective outs must be offset-0 full tensors:
                        # sliced outputs silently corrupt on HW)
                        p = ci * 2 + t // 2
                        nc.gpsimd.collective_compute(
                            "AllGather", mybir.AluOpType.bypass,
                            replica_groups=[list(range(NCORE))],
                            ins=[d['h_loc'][PLO[p]:PHI[p], :]],
                            outs=[d['h_full%d' % p][:, :]])

        # ---------------- SAGE scatter (+ deferred AG piece 2) -----------
        # SBUF pools for G/Pt/idx hoisted to the outer scope (aliasing
        # attention tiles would delay the gathers to attention end).
        aggb = main.tile([F, IPAD], bf16, name="aggb")
        hT = main.tile([F, IPAD], bf16, name="hT")
        idx_t = sin.tile([128, CH * 8], mybir.dt.int16, name="idx_t")
        nc.sync.dma_start(out=idx_t[:], in_=d['gidx'][:, :])
        recT_t = sin.tile([F, IPAD], f32, name="recT_t")
        nc.sync.dma_start(out=recT_t[:], in_=d['recipT'][:, :])
        At = main.tile([128, NGRP * DBLK * 512], bf16, name="At")
        if BIS != 13:
            for g in range(NGRP):
                nc.sync.dma_start(
                    out=At[:, g * DBLK * 512:(g + 1) * DBLK * 512],
                    in_=d['Aloc'][:, g * DBLK * 512:(g + 1) * DBLK * 512])
        with tc.tile_pool(name="scp", bufs=4, space="PSUM") as scp, \
             tc.tile_pool(name="htp", bufs=2, space="PSUM") as htp:
            GH = (max(NCH) + NSPL - 1) // NSPL + 1
            # Gate all SAGE PE work behind the last attention matmul (the
            # scheduler's dma_gather cost model is wrong; ungated SAGE PE
            # head-of-line blocks the attention FIFO, and the PSUM banks
            # alias attention pools anyway).
            first_mm = [None]

            def gate(inst):
                if first_mm[0] is None:
                    add_dep_helper(inst.ins, last_att_mm.ins,
                                   reason="SAGE PE after attention")
                    first_mm[0] = inst

            def spans_of(p):
                qs = [NCH[p] * k // NSPL for k in range(NSPL + 1)]
                return qs, [(qs[k], qs[k + 1]) for k in range(NSPL)]

            Gmap = {}

            def gather_call(p, si):
                qs, spans = spans_of(p)
                c0, c1 = spans[si]
                G = gat.tile([128, GH, HPAD], bf16, tag="G", name="G")
                nc.gpsimd.dma_gather(
                    out_ap=G[:, :c1 - c0, :],
                    in_ap=d['h_full%d' % p][:, :],
                    idxs_ap=idx_t[:, (POFF[p] + c0) * 8:
                                  (POFF[p] + c1) * 8],
                    num_idxs=(c1 - c0) * 128,
                    num_idxs_reg=(c1 - c0) * 128,
                    elem_size=HPAD,
                    single_packet=False)
                # Pt slice for this call span (keeps SBUF down; P is
                # stored per piece so the DRAM row stride stays <64KB)
                Pt = pin.tile([128, GH * 512], bf16, tag="P", name="Pt")
                nc.sync.dma_start(
                    out=Pt[:, :(c1 - c0) * 512],
                    in_=d['P%d' % p][:, c0 * 512:c1 * 512])
                Gmap[(p, si)] = (G, Pt, c0, c1)

            def piece(p):
                qs, spans = spans_of(p)
                Gs = [Gmap[(p, si)] for si in range(NSPL)]
                ch = 0
                for g in range(NGRP):
                    if S_pg[p][g] == 0:
                        continue
                    # two psum tiles, both at partition base 0: half h of
                    # each chunk runs on quadrant (row 64h, col 0) -> DVE
                    # can consume both without a partition move
                    accA = scp.tile([F, 512], f32, space="PSUM", tag="agg",
                                    name="accpA")
                    accB = scp.tile([F, 512], f32, space="PSUM", tag="agg",
                                    name="accpB")
                    for s in range(S_pg[p][g]):
                        gsel = 0
                        while ch >= qs[gsel + 1]:
                            gsel += 1
                        G, Pt, c0, c1 = Gs[gsel]
                        for h in range(2):
                            mi = nc.tensor.matmul(
                                out=(accA if h == 0 else accB)[:, :],
                                lhsT=G[64 * h:64 * h + 64, ch - c0, :F],
                                rhs=Pt[64 * h:64 * h + 64,
                                       (ch - c0) * 512:(ch - c0 + 1) * 512],
                                start=(s == 0), stop=(s == S_pg[p][g] - 1),
                                skip_group_check=True)
                            gate(mi)
                        ch += 1
                    sl = aggS[:, g * 512:(g + 1) * 512]
                    nc.vector.tensor_add(out=sl, in0=sl, in1=accA[:, :])
                    nc.vector.tensor_add(out=sl, in0=sl, in1=accB[:, :])
                    if p == NP - 1:
                        nc.vector.tensor_mul(
                            out=aggb[:, g * 512:(g + 1) * 512],
                            in0=aggS[:, g * 512:(g + 1) * 512],
                            in1=recT_t[:, g * 512:(g + 1) * 512])

            # self-core edges: dense local group adjacency from hnat
            # (runs right at attention end, hidden under the gathers)
            for g in range(NGRP if BIS not in (12, 13) else 0):
                accA = scp.tile([F, 512], f32, space="PSUM", tag="agg",
                                name="accselfA")
                accB = scp.tile([F, 512], f32, space="PSUM", tag="agg",
                                name="accselfB")
                for sb in range(DBLK):
                    for h in range(2):
                        mi = nc.tensor.matmul(
                            out=(accA if h == 0 else accB)[:, :],
                            lhsT=hnat[64 * h:64 * h + 64, sb, :F],
                            rhs=At[64 * h:64 * h + 64,
                                   (g * DBLK + sb) * 512:
                                   (g * DBLK + sb + 1) * 512],
                            start=(sb == 0), stop=(sb == DBLK - 1),
                            skip_group_check=True)
                        gate(mi)
                sl = aggS[:, g * 512:(g + 1) * 512]
                nc.vector.tensor_add(out=sl, in0=sl, in1=accA[:, :])
                nc.vector.tensor_add(out=sl, in0=sl, in1=accB[:, :])
            if BIS in (11, 12, 13):
                nc.vector.memset(aggb[:], 0.0)
            if BIS not in (11, 12, 13):
                # Gather-call emission order: GpSimd dispatches these in
                # order, so placing piece 4's first call right after piece
                # 3's first gives AG4 a gather-free window on the DMA
                # engines (the call head-blocks the queue while waiting on
                # the AG4 semaphore, pausing gather traffic).
                ORDER = [(0, 0), (0, 1), (1, 0), (1, 1), (2, 0), (2, 1),
                         (3, 0), (4, 0), (3, 1), (4, 1), (5, 0), (5, 1)]
                for (p, si) in ORDER:
                    gather_call(p, si)
                piece(0)
            # hT (bf16) for SAGE lin_r: transpose the 12 h tiles (PE work
            # that fills the gap while gathers run on GpSimd)
            for t in range(DBLK):
                pst = htp.tile([F, 128], bf16, space="PSUM", tag="ht",
                               name="psht")
                ti = nc.tensor.transpose(out=pst[:], in_=hnat[:, t, :F],
                                         identity=ident_t[:])
                if t == 0:
                    add_dep_helper(ti.ins, last_att_mm.ins,
                                   reason="transposes after attention")
                nc.vector.tensor_copy(out=hT[:, t * 128:(t + 1) * 128],
                                      in_=pst[:])
            if BIS not in (11, 12, 13):
                for p in range(1, NP):
                    piece(p)

        # ---------------- SAGE linear + pool + MLP ----------------
        with tc.tile_pool(name="mlpw", bufs=1) as mlpw, \
             tc.tile_pool(name="mlps", bufs=2) as mlps, \
             tc.tile_pool(name="mlpp", bufs=2, space="PSUM") as mlpp:
            WllT_t = mlpw.tile([F, F], bf16, name="WllT_t")
            nc.sync.dma_start(out=WllT_t[:], in_=d['WllT'][:, :])
            WlrT_t = mlpw.tile([F, F], bf16, name="WlrT_t")
            nc.sync.dma_start(out=WlrT_t[:], in_=d['WlrT'][:, :])
            bll_t = mlpw.tile([F, 1], f32, name="bll_t")
            nc.sync.dma_start(out=bll_t[:], in_=d['bll'][:, :])
            Wg1T_t = mlpw.tile([F, 1500], f32, name="Wg1T_t")
            nc.sync.dma_start(out=Wg1T_t[:], in_=d['Wg1T'][:, :])
            bg1_t = mlpw.tile([128, 12], f32, name="bg1_t")
            nc.sync.dma_start(out=bg1_t[:], in_=d['bg1'][:, :])
            Wg2_t = mlpw.tile([128, 12 * 128], f32, name="Wg2_t")
            nc.sync.dma_start(out=Wg2_t[:], in_=d['Wg2Tr'][:, :])
            bg2_t = mlpw.tile([128, 1], f32, name="bg2_t")
            nc.sync.dma_start(out=bg2_t[:], in_=d['bg2'][:, :])
            WoT_t = mlpw.tile([128, 1], f32, name="WoT_t")
            nc.sync.dma_start(out=WoT_t[:], in_=d['WoT'][:, :])

            relu_f = mybir.ActivationFunctionType.Relu
            h2T = mlps.tile([F, IPAD], f32, tag="h2T", name="h2T")
            for cc in range(NGRP):
                ps = mlpp.tile([F, 512], f32, space="PSUM", tag="h2",
                               name="psh2")
                nc.tensor.matmul(out=ps[:], lhsT=WllT_t[:],
                                 rhs=aggb[:, cc * 512:(cc + 1) * 512],
                                 start=True, stop=False,
                                 skip_group_check=True)
                nc.tensor.matmul(out=ps[:], lhsT=WlrT_t[:],
                                 rhs=hT[:, cc * 512:(cc + 1) * 512],
                                 start=False, stop=True,
                                 skip_group_check=True)
                nc.scalar.activation(out=h2T[:, cc * 512:(cc + 1) * 512],
                                     in_=ps[:], func=relu_f, bias=bll_t[:])

            gT = mlps.tile([F, GB], f32, tag="gT", name="gT")
            for g in range(GB):
                lo, hi = GRAPH_BOUNDS[g], GRAPH_BOUNDS[g + 1]
                nc.vector.tensor_reduce(out=gT[:, g:g + 1], in_=h2T[:, lo:hi],
                                        axis=mybir.AxisListType.X,
                                        op=mybir.AluOpType.max)
            g1T = mlps.tile([128, 12, GB], f32, tag="g1T", name="g1T")
            for j in range(12):
                w = min(128, 1500 - j * 128)
                ps = mlpp.tile([128, GB], f32, space="PSUM", tag="g1",
                               name="psg1")
                nc.tensor.matmul(out=ps[:w, :],
                                 lhsT=Wg1T_t[:, j * 128:j * 128 + w],
                                 rhs=gT[:], start=True, stop=True)
                if w < 128:
                    nc.vector.memset(g1T[:, j, :], 0.0)
                nc.scalar.activation(out=g1T[:w, j, :], in_=ps[:w, :],
                                     func=relu_f, bias=bg1_t[:w, j:j + 1])
            g2ps = mlpp.tile([128, GB], f32, space="PSUM", tag="g2",
                             name="g2ps")
            for j in range(12):
                nc.tensor.matmul(out=g2ps[:],
                                 lhsT=Wg2_t[:, j * 128:(j + 1) * 128],
                                 rhs=g1T[:, j, :], start=(j == 0),
                                 stop=(j == 11), skip_group_check=True)
            g2sb = mlps.tile([128, GB], f32, tag="g2sb", name="g2sb")
            nc.vector.tensor_scalar_add(out=g2sb[:], in0=g2ps[:],
                                        scalar1=bg2_t[:])
            ops = mlpp.tile([1, GB], f32, space="PSUM", tag="o", name="ops")
            nc.tensor.matmul(out=ops[:], lhsT=WoT_t[:], rhs=g2sb[:],
                             start=True, stop=True)
            osb = mlps.tile([1, GB], f32, tag="osb", name="osb")
            nc.vector.tensor_scalar_add(out=osb[:], in0=ops[:],
                                        scalar1=float(bo_const))
            nc.sync.dma_start(out=d['out8'][:, :], in_=osb[:])


def _build_program(S_pg, bo_const):
    import concourse.tile as tile
    from concourse import bacc, mybir

    f32 = mybir.dt.float32
    bf16 = mybir.dt.bfloat16
    CH = int(sum(S_pg))
    nc = bacc.Bacc("TRN2", target_bir_lowering=False, debug=False,
                   num_devices=NCORE)

    d = {}

    def dram_in(name, shape, dt=f32):
        d[name] = nc.dram_tensor(name, list(shape), dt, kind="ExternalInput")

    dram_in("xh", (F1, XW), bf16)
    dram_in("xn", (128, JT * F1), bf16)
    dram_in("KQ", (F1, IPAD), bf16)
    dram_in("Wva", (F1, F1), bf16)
    dram_in("Vl", (128, DBLK * F), f32)
    dram_in("ident", (128, 128), bf16)
    dram_in("WllT", (F, F), bf16)
    dram_in("WlrT", (F, F), bf16)
    dram_in("bll", (F, 1))
    dram_in("Wg1T", (F, 1500))
    dram_in("bg1", (128, 12))
    dram_in("Wg2Tr", (128, 12 * 128))
    dram_in("bg2", (128, 1))
    dram_in("WoT", (128, 1))
    dram_in("recipT", (F, IPAD))
    NCH = [int(sum(S_pg[p * NGRP:(p + 1) * NGRP])) for p in range(NP)]
    for p in range(NP):
        dram_in("P%d" % p, (128, NCH[p] * 512), bf16)
    dram_in("Aloc", (128, NGRP * DBLK * 512), bf16)
    d['gidx'] = nc.dram_tensor("gidx", [128, CH * 8], mybir.dt.int16,
                               kind="ExternalInput")
    d['out8'] = nc.dram_tensor("out8", [1, GB], f32, kind="ExternalOutput")
    d['h_loc'] = nc.dram_tensor("h_loc", [ROWS, HPAD], bf16)
    for p in range(NP):
        d['h_full%d' % p] = nc.dram_tensor(
            "h_full%d" % p, [NCORE * PLEN[p], HPAD], bf16,
            addr_space="Shared")

    with tile.TileContext(nc) as tc:
        _emit_body(nc, tc, d, S_pg, bo_const)

    nc.compile()
    return nc


# --------------------------------------------------------------------------
# entry point
# --------------------------------------------------------------------------

_CACHE = {}


def _make_in_maps(inputs):
    x = np.asarray(inputs['x'], np.float32)
    edge_index = np.asarray(inputs['edge_index'])
    w = _prep_weights(inputs)
    xh, xn, KQ = _prep_x(x, w['M'])
    Vl = _prep_vl(x, inputs)
    gidx, Ps, Als, recipT, S_pg = _prep_edges(edge_index)
    ident = np.eye(128, dtype=BF16)
    common = dict(
        xh=xh, xn=xn, Wva=w['Wva'], ident=ident,
        WllT=w['WllT'], WlrT=w['WlrT'],
        bll=w['bll'], Wg1T=w['Wg1T'], bg1=w['bg1'], Wg2Tr=w['Wg2Tr'],
        bg2=w['bg2'], WoT=w['WoT'])
    in_maps = []
    S_pg2 = [list(S_pg[p * NGRP:(p + 1) * NGRP]) for p in range(NP)]
    NCH = [int(sum(S_pg2[p])) for p in range(NP)]
    POFF = [int(sum(NCH[:p])) for p in range(NP)]
    for c in range(NCORE):
        m = dict(common)
        m['KQ'] = KQ[c]
        m['Vl'] = Vl[c]
        m['gidx'] = gidx[c]
        for p in range(NP):
            m['P%d' % p] = np.ascontiguousarray(
                Ps[c][:, POFF[p] * 512:(POFF[p] + NCH[p]) * 512])
        m['Aloc'] = Als[c]
        m['recipT'] = recipT[c]
        in_maps.append(m)
    return in_maps, S_pg, w['bo']


def kernel(**inputs):
    from concourse.bass_utils import run_bass_kernel_spmd

    import os
    in_maps, S_pg, bo = _make_in_maps(inputs)
    key = ('prog', S_pg, bo, os.environ.get('KBISECT', '0'))
    if key not in _CACHE:
        _CACHE[key] = _build_program(S_pg, bo)
    nc = _CACHE[key]

    res = run_bass_kernel_spmd(nc, in_maps, list(range(NCORE)))
    global LAST_RESULT
    LAST_RESULT = res
    out = np.zeros((B, 1), np.float32)
    for c in range(NCORE):
        out[c * GB:(c + 1) * GB, 0] = res.results[c]['out8'].reshape(-1)
    return out


LAST_RESULT = None


# revision 78
# speedup vs baseline: 1.0196x; 1.0196x over previous
"""Trainium2 Bass kernel for nn_GAT_GraphSAGE (N=12000, E=192000, F=35, B=64).

Sharding: attention rows (softmax row i = K_new index) sharded 1500/core on
8 cores; one AllGather of post-attention h per 512-row piece (bf16); SAGE
sharded by dst with a batched dma_gather of h[src] rows + one-hot-matmul
scatter; per-core global-max-pool + MLP head on that core's 8 graphs.

v4 = v2 attention (3 i-pieces of 512, exp on ACT is the in-attention
bottleneck) + restructured SAGE scatter:
- Gather slots deduplicated per (src, piece, 512-col dst group) and packed
  into full 128-slot chunks (24448 -> 16384 slots; GpSimd SWDGE descriptor
  generation measures ~7ns/slot on HW, and is the serial bottleneck after
  attention ends).
- Each chunk scatters via a [64,512] one-hot matmul pair: top/bottom
  64-slot halves write opposite psum partition halves, alternating per
  chunk so all 4 PE quadrants stay busy; the two halves are summed into
  aggS once per (piece, group).
- Self-core edges keep the dense local block adjacency, restructured to
  512-col dst groups with the same quadrant packing.
- MLP head weights in bf16 (kills the fp32 LOW/HIGH double-pass matmuls
  in the tail).
"""
import math
import numpy as np
import ml_dtypes

BF16 = ml_dtypes.bfloat16

N, E, F, B = 12000, 192000, 35, 64
F1 = F + 1
NCORE = 8
ROWS = N // NCORE            # 1500
ICH = 512
NI = 3
IPAD = ICH * NI              # 1536
JT = 94                      # j chunks of 128
JPAD = JT * 128              # 12032
XW = 12064                   # padded x~^T width (covers 7*1500 + 1536)
DBLK = 12                    # dst blocks (128 each) per core
NGRP = 3                     # dst groups of 512 cols
GB = B // NCORE              # 8 graphs per core
HPAD = 128                   # h row padded to 128 bf16 (256B) for dma_gather
GRAPH_BOUNDS = [int(math.ceil(g * (N / B))) for g in range(GB + 1)]
# AllGather pieces: 5 of 2 h-blocks, then the last two blocks separately
# (128 and 92 rows) so the latest-arriving attention output gates as
# little gather work as possible.
NP = 7
PLO = [0, 256, 512, 768, 1024, 1280, 1408]
PHI = [256, 512, 768, 1024, 1280, 1408, 1500]
PLEN = [PHI[p] - PLO[p] for p in range(NP)]
NSPL = 2                     # gather calls per piece (1 for the last two)


# --------------------------------------------------------------------------
# host-side preprocessing
# --------------------------------------------------------------------------

def _prep_weights(p):
    f64 = np.float64
    f32 = np.float32
    Wq, bq = p['Wq'].astype(f64), p['bq'].astype(f64)
    Wk, bk = p['Wk'].astype(f64), p['bk'].astype(f64)
    Wv, bv = p['Wv'].astype(f64), p['bv'].astype(f64)
    W3c, b3 = p['W3'][:, :, 1].astype(f64), p['b3'].astype(f64)
    W5c, b5 = p['W5'][:, :, 2].astype(f64), p['b5'].astype(f64)
    Wl, bl = p['Wl'].astype(f64), p['bl'].astype(f64)
    Wl1, Wl2, Wl3 = Wl[:, :F], Wl[:, F:2 * F], Wl[:, 2 * F:]

    # K_new = x~ @ Wkn~  (F1 -> F affine, includes 1/sqrt(F))
    Weff = W3c.T @ Wl1.T + W5c.T @ Wl2.T + Wl3.T
    beff = b3 @ Wl1.T + b5 @ Wl2.T + bl
    Wkn = Wk.T @ Weff
    bkn = bk @ Weff + beff
    s = 1.0 / np.sqrt(F)
    Wkn_aug = np.vstack([Wkn, bkn[None, :]]) * s          # [F1, F]
    Wq_aug = np.vstack([Wq.T, bq[None, :]])               # [F1, F]
    M = Wkn_aug @ Wq_aug.T                                # [F1, F1]

    Wva = np.zeros((F1, F1))
    Wva[:F, :F] = Wv.T
    Wva[F, :F] = bv
    Wva[F, F] = 1.0                                       # denominator column
    out = {'M': M, 'Wva': Wva.astype(BF16)}
    out['WllT'] = np.ascontiguousarray(p['Wll'].T).astype(BF16)
    out['WlrT'] = np.ascontiguousarray(p['Wlr'].T).astype(BF16)
    out['bll'] = p['bll'].astype(f32).reshape(F, 1)
    out['Wg1T'] = np.ascontiguousarray(p['Wg1'].T).astype(f32)   # [35,1500]
    bg1 = np.zeros((128, 12), f32)
    bg1.T.reshape(-1)[:1500] = p['bg1'].astype(f32)
    out['bg1'] = bg1
    w2 = np.zeros((12 * 128, 128), f32)
    w2[:1500, :] = p['Wg2'].T.astype(f32)
    out['Wg2Tr'] = np.ascontiguousarray(
        w2.reshape(12, 128, 128).transpose(1, 0, 2).reshape(128, 12 * 128))
    out['bg2'] = p['bg2'].astype(f32).reshape(128, 1)
    out['WoT'] = p['Wo'].astype(f32).reshape(1, 128).T.copy()
    out['bo'] = float(np.asarray(p['bo']).reshape(-1)[0])
    return out


def _prep_x(x, M):
    """Host: x~^T (bf16), x~ natural chunked (bf16), per-core KQT."""
    x64 = np.asarray(x, np.float64)
    xa = np.concatenate([x64, np.ones((N, 1))], axis=1)       # [N, F1]
    xaT = np.zeros((F1, XW))
    xaT[:, :N] = xa.T                                         # pad cols zero
    xh = xaT.astype(BF16)                                     # [F1, XW]

    # natural chunks for the U accumulation: [128, JT, F1]
    xn = np.zeros((128, JT, F1))
    flat = xaT[:, :JPAD].T                                    # [JPAD, F1]
    xn[:, :, :] = flat.reshape(JT, 128, F1).transpose(1, 0, 2)
    xn = np.ascontiguousarray(xn.reshape(128, JT * F1)).astype(BF16)

    KQ = []
    for c in range(NCORE):
        sl = xaT[:, c * ROWS: c * ROWS + IPAD]                # [F1, IPAD]
        KQ.append(np.ascontiguousarray(M.T @ sl).astype(BF16))
    return xh, xn, KQ


def _prep_vl(x, p):
    """Per-core natural V' local [128, DBLK*F] f32 (for the residual)."""
    f64 = np.float64
    Wv, bv = p['Wv'].astype(f64), p['bv'].astype(f64)
    x64 = np.asarray(x, np.float64)
    V = x64 @ Wv.T + bv                                       # [N, F]
    out = []
    for c in range(NCORE):
        vl = np.zeros((DBLK * 128, F))
        vl[:ROWS] = V[c * ROWS:(c + 1) * ROWS]
        out.append(np.ascontiguousarray(
            vl.reshape(DBLK, 128, F).transpose(1, 0, 2).reshape(128, DBLK * F)
        ).astype(np.float32))
    return out


def _prep_edges(edge_index):
    """Non-self edges keyed (piece p of src, dst group g of 512 cols),
    DEDUPED per (src, p, g): each gathered slot is a unique src row whose
    P columns cover every dst it feeds in that group.  Chunk stream is
    p-major then g-major with S_pg (global max over cores) 128-slot chunks
    per (p, g).  Self-core edges go to a dense local block adjacency Aloc
    laid out per (group, src block).
    """
    src = np.asarray(edge_index[0], np.int64)
    dst = np.asarray(edge_index[1], np.int64)
    deg = np.bincount(dst, minlength=N).astype(np.float64)
    recip = (1.0 / np.maximum(deg, 1.0)).astype(np.float32)

    core_of = dst // ROWS
    dloc = dst - core_of * ROWS
    grp_of = dloc // 512
    sc = src // ROWS
    sr = src - sc * ROWS
    piece_of = np.searchsorted(np.array(PHI), sr, side='right')
    piece_of = np.minimum(piece_of, NP - 1)
    selfm = sc == core_of
    ns = ~selfm

    # ---- self edges: dense [128, NGRP * DBLK * 512] block adjacency ----
    Als = []
    for c in range(NCORE):
        m = selfm & (core_of == c)
        A = np.zeros((128, NGRP * DBLK * 512), np.float32)
        ssb = sr[m] // 128              # src block
        ssp = sr[m] - ssb * 128         # src pos in block
        g = grp_of[m]
        rel = dloc[m] - g * 512
        np.add.at(A, (ssp, (g * DBLK + ssb) * 512 + rel), 1.0)
        Als.append(np.ascontiguousarray(A.astype(BF16)))

    # ---- non-self: unique (core, piece, grp, src) slots ----
    key = ((core_of * NP + piece_of) * NGRP + grp_of) * N + src
    ukey, inv = np.unique(key[ns], return_inverse=True)
    ucpg = ukey // N
    usrc = ukey - ucpg * N
    uc = ucpg // (NP * NGRP)
    up = (ucpg // NGRP) % NP
    ug = ucpg % NGRP
    counts = np.zeros((NCORE, NP, NGRP), np.int64)
    np.add.at(counts, (uc, up, ug), 1)
    S_pg = np.ceil(counts.max(axis=0) / 128).astype(np.int64)  # [NP, NGRP]
    CH = int(S_pg.sum())

    ch_off = np.zeros((NP, NGRP), np.int64)
    acc = 0
    for p in range(NP):
        for g in range(NGRP):
            ch_off[p, g] = acc
            acc += S_pg[p, g]

    # slot position within its (c,p,g) cell (ukey sorted -> contiguous)
    cell_id = ucpg
    cell_starts = np.searchsorted(cell_id, np.arange(NCORE * NP * NGRP))
    slot_in_cell = np.arange(len(ukey)) - cell_starts[cell_id]
    gslot = ch_off[up, ug] * 128 + slot_in_cell

    # gather position within h_full_p
    plen_arr = np.array(PLEN)[up]
    plo_arr = np.array(PLO)[up]
    su_c = usrc // ROWS
    su_r = usrc - su_c * ROWS
    pos = su_c * plen_arr + (su_r - plo_arr)

    gidx, Ps = [], []
    for c in range(NCORE):
        mu = uc == c
        idx_c = np.zeros(CH * 128, np.int16)
        idx_c[gslot[mu]] = pos[mu].astype(np.int16)
        gidx.append(np.ascontiguousarray(
            np.tile(idx_c.reshape(-1, 16).T, (8, 1))))
        Ps.append(np.zeros((128, CH * 512), np.float32))

    # fill P: each non-self edge scatters its unique slot to its rel col
    e_slot = gslot[inv]
    e_core = core_of[ns]
    e_rel = dloc[ns] - grp_of[ns] * 512
    e_chunk = e_slot // 128
    e_sp = e_slot - e_chunk * 128
    for c in range(NCORE):
        m = e_core == c
        np.add.at(Ps[c], (e_sp[m], e_chunk[m] * 512 + e_rel[m]), 1.0)
    Ps = [np.ascontiguousarray(P.astype(BF16)) for P in Ps]

    recipT = []
    for c in range(NCORE):
        r = np.ones(IPAD, np.float32)
        r[:ROWS] = recip[c * ROWS:(c + 1) * ROWS]
        recipT.append(np.ascontiguousarray(np.broadcast_to(r, (F, IPAD))))
    return gidx, Ps, Als, recipT, tuple(int(v) for v in S_pg.reshape(-1))


# --------------------------------------------------------------------------
# device program
# --------------------------------------------------------------------------

def _emit_body(nc, tc, d, S_pg, bo_const):
    import concourse.tile as tile
    import os
    from concourse import mybir
    from concourse.tile import add_dep_helper

    BIS = int(os.environ.get('KBISECT', '0'))
    f32 = mybir.dt.float32
    bf16 = mybir.dt.bfloat16
    S_pg = [list(S_pg[p * NGRP:(p + 1) * NGRP]) for p in range(NP)]
    NCH = [int(sum(S_pg[p])) for p in range(NP)]       # chunks per piece
    CH = sum(NCH)
    POFF = [int(sum(NCH[:p])) for p in range(NP)]      # piece chunk offsets

    with tc.tile_pool(name="const", bufs=1) as constp, \
         tc.tile_pool(name="main", bufs=1) as main, \
         tc.tile_pool(name="gat", bufs=12) as gat, \
         tc.tile_pool(name="pin", bufs=3) as pin, \
         tc.tile_pool(name="sin", bufs=1) as sin:
        # ---- inputs (leading xhT slices + xn first so exp starts early) ----
        KQT = main.tile([128, IPAD], bf16, name="KQT")
        nc.sync.dma_start(out=KQT[0:F1, :], in_=d['KQ'][:, :])
        nc.sync.dma_start(out=KQT[64:64 + F1, :], in_=d['KQ'][:, :])
        xhT = main.tile([128, XW], bf16, name="xhT")
        xn = main.tile([128, JT * F1], bf16, name="xn")
        HW = XW // 8
        XNW = (JT * F1) // 4
        for q in range(8):
            nc.sync.dma_start(out=xhT[0:F1, q * HW:(q + 1) * HW],
                              in_=d['xh'][:, q * HW:(q + 1) * HW])
            nc.sync.dma_start(out=xhT[64:64 + F1, q * HW:(q + 1) * HW],
                              in_=d['xh'][:, q * HW:(q + 1) * HW])
            if q < 4:
                nc.sync.dma_start(out=xn[:, q * XNW:(q + 1) * XNW],
                                  in_=d['xn'][:, q * XNW:(q + 1) * XNW])
        Wva_t = constp.tile([F1, F1], bf16, name="Wva_t")
        nc.sync.dma_start(out=Wva_t[:], in_=d['Wva'][:, :])
        Vl = main.tile([128, DBLK * F], f32, name="Vl")
        nc.sync.dma_start(out=Vl[:], in_=d['Vl'][:, :])
        ident_t = constp.tile([128, 128], bf16, name="ident_t")
        nc.sync.dma_start(out=ident_t[:], in_=d['ident'][:, :])

        hnat = main.tile([128, DBLK, HPAD], bf16, name="hnat")
        nc.vector.memset(hnat[:, :, F:HPAD], 0.0)
        aggS = main.tile([F, IPAD], f32, name="aggS")
        nc.vector.memset(aggS[:], 0.0)

        # ---------------- attention ----------------
        # groups of 2 j-chunks; one [128,1024] exp per group (2 PSUM banks,
        # double-buffered). U' = sum_j x~_j^T exp[j,:] accumulated in two
        # K-half chains (row groups 0/64 -> UC/UD); V-projection after.
        # UC/UD double-buffered (Up bufs=2) so piece ci+1's U-chain starts
        # without waiting for ci's h post-processing.
        GROUPS = [(g * 2, min(2, JT - g * 2)) for g in range((JT + 1) // 2)]
        exp_f = mybir.ActivationFunctionType.Exp
        with tc.tile_pool(name="mm1p", bufs=2, space="PSUM") as mm1p, \
             tc.tile_pool(name="Up", bufs=2, space="PSUM") as Upp, \
             tc.tile_pool(name="esb", bufs=3) as esb, \
             tc.tile_pool(name="usb", bufs=2) as usb, \
             tc.tile_pool(name="hsm", bufs=4) as hsmall:
            for ci in range(NI):
                UC = Upp.tile([128, ICH], f32, tag="uc", name="UC")
                UD = Upp.tile([128, ICH], f32, tag="ud", name="UD")
                prev = None
                for (j0, glen) in GROUPS:
                    ps = mm1p.tile([128, 2 * ICH], f32, space="PSUM",
                                   tag="s", name="pss")
                    for k in range(glen):
                        j = j0 + k
                        r = 64 * (j & 1)
                        for ch in range(2):
                            nc.tensor.matmul(
                                out=ps[64 * ch:64 * ch + 64,
                                       k * ICH:(k + 1) * ICH],
                                lhsT=xhT[r:r + F1,
                                         j * 128 + 64 * ch:
                                         j * 128 + 64 * ch + 64],
                                rhs=KQT[r:r + F1,
                                        ci * ICH:(ci + 1) * ICH],
                                start=True, stop=True)
                    et = esb.tile([128, 3 * ICH], bf16, tag="e", name="et")
                    nc.scalar.activation(out=et[:, :glen * ICH],
                                         in_=ps[:, :glen * ICH], func=exp_f)
                    if prev is not None:
                        pe, pj0, pglen = prev
                        for k in range(pglen):
                            j = pj0 + k
                            for r in range(2):
                                nc.tensor.matmul(
                                    out=(UC if r == 0 else UD)[0:F1, :],
                                    lhsT=xn[64 * r:64 * r + 64,
                                            j * F1:(j + 1) * F1],
                                    rhs=pe[64 * r:64 * r + 64,
                                           k * ICH:(k + 1) * ICH],
                                    start=(j == 0), stop=False,
                                    skip_group_check=True)
                    prev = (et, j0, glen)
                pe, pj0, pglen = prev
                for k in range(pglen):
                    j = pj0 + k
                    for r in range(2):
                        nc.tensor.matmul(
                            out=(UC if r == 0 else UD)[0:F1, :],
                            lhsT=xn[64 * r:64 * r + 64, j * F1:(j + 1) * F1],
                            rhs=pe[64 * r:64 * r + 64,
                                   k * ICH:(k + 1) * ICH],
                            start=False, stop=(k == pglen - 1),
                            skip_group_check=True)
                # combine K-halves -> U'sb bf16 [F1, 512]
                # (avoid a two-PSUM-operand tensor_tensor: copy then add)
                Ucs = usb.tile([F1, ICH], f32, tag="ucs", name="Ucs")
                nc.vector.tensor_copy(out=Ucs[:], in_=UC[0:F1, :])
                Usb = usb.tile([F1, ICH], bf16, tag="usb", name="Usb")
                nc.vector.tensor_add(out=Usb[:], in0=Ucs[:],
                                     in1=UD[0:F1, :])
                # h natural: hraw[i,g] = sum_f U'sb[f,i] Wva[f,g]
                # (two 64-col halves to stay in the 64x64 tile grid)
                for t in range(4):
                    blk = ci * 4 + t
                    for ch in range(2):
                        last_att_mm = nc.tensor.matmul(
                            out=UD[64 * ch:64 * ch + 64,
                                   t * 128:t * 128 + F1],
                            lhsT=Usb[:, t * 128 + 64 * ch:
                                     t * 128 + 64 * ch + 64],
                            rhs=Wva_t[:],
                            start=True, stop=True, skip_group_check=True)
                    hraw = UD[:, t * 128:t * 128 + F1]
                    rec = hsmall.tile([128, 1], f32, tag="rec", name="rec")
                    nc.vector.reciprocal(out=rec[:], in_=hraw[:, F:F1])
                    hh = hsmall.tile([128, F], f32, tag="hh", name="hh")
                    nc.vector.scalar_tensor_tensor(
                        out=hh[:], in0=hraw[:, :F], scalar=rec[:],
                        in1=Vl[:, blk * F:(blk + 1) * F],
                        op0=mybir.AluOpType.mult,
                        op1=mybir.AluOpType.add)
                    nc.vector.tensor_scalar_max(out=hnat[:, blk, :F],
                                                in0=hh[:], scalar1=0.0)
                    lo = blk * 128
                    nrows = min(128, max(0, ROWS - lo))
                    if nrows > 0:
                        nc.sync.dma_start(
                            out=d['h_loc'][lo:lo + nrows, :],
                            in_=hnat[:nrows, blk, :])
                    # AG a piece as soon as its blocks are all out
                    # (collective outs must be offset-0 full tensors:
                    # sliced outputs silently corrupt on HW)
                    pt = ({1: 0, 3: 1, 5: 2, 7: 3, 9: 4, 10: 5, 11: 6}
                          .get(blk))
                    if pt is not None:
                        nc.gpsimd.collective_compute(
                            "AllGather", mybir.AluOpType.bypass,
                            replica_groups=[list(range(NCORE))],
                            ins=[d['h_loc'][PLO[pt]:PHI[pt], :]],
                            outs=[d['h_full%d' % pt][:, :]])

        # ---------------- SAGE scatter (+ deferred AG piece 2) -----------
        # SBUF pools for G/Pt/idx hoisted to the outer scope (aliasing
        # attention tiles would delay the gathers to attention end).
        aggb = main.tile([F, IPAD], bf16, name="aggb")
        hT = main.tile([F, IPAD], bf16, name="hT")
        idx_t = sin.tile([128, CH * 8], mybir.dt.int16, name="idx_t")
        nc.sync.dma_start(out=idx_t[:], in_=d['gidx'][:, :])
        recT_t = sin.tile([F, IPAD], f32, name="recT_t")
        nc.sync.dma_start(out=recT_t[:], in_=d['recipT'][:, :])
        At = main.tile([128, NGRP * DBLK * 512], bf16, name="At")
        if BIS != 13:
            for g in range(NGRP):
                nc.sync.dma_start(
                    out=At[:, g * DBLK * 512:(g + 1) * DBLK * 512],
                    in_=d['Aloc'][:, g * DBLK * 512:(g + 1) * DBLK * 512])
        with tc.tile_pool(name="scp", bufs=4, space="PSUM") as scp, \
             tc.tile_pool(name="htp", bufs=2, space="PSUM") as htp:
            NSPL_CAP = [NSPL] * 5 + [1, 1]
            GH = max((NCH[p] + NSPL_CAP[p] - 1) // NSPL_CAP[p]
                     for p in range(NP)) + 1
            # Gate all SAGE PE work behind the last attention matmul (the
            # scheduler's dma_gather cost model is wrong; ungated SAGE PE
            # head-of-line blocks the attention FIFO, and the PSUM banks
            # alias attention pools anyway).
            first_mm = [None]

            def gate(inst):
                if first_mm[0] is None:
                    add_dep_helper(inst.ins, last_att_mm.ins,
                                   reason="SAGE PE after attention")
                    first_mm[0] = inst

            NSPL_P = NSPL_CAP

            def spans_of(p):
                ns = NSPL_P[p]
                qs = [NCH[p] * k // ns for k in range(ns + 1)]
                return qs, [(qs[k], qs[k + 1]) for k in range(ns)]

            Gmap = {}

            def gather_call(p, si):
                qs, spans = spans_of(p)
                c0, c1 = spans[si]
                G = gat.tile([128, GH, HPAD], bf16, tag="G", name="G")
                nc.gpsimd.dma_gather(
                    out_ap=G[:, :c1 - c0, :],
                    in_ap=d['h_full%d' % p][:, :],
                    idxs_ap=idx_t[:, (POFF[p] + c0) * 8:
                                  (POFF[p] + c1) * 8],
                    num_idxs=(c1 - c0) * 128,
                    num_idxs_reg=(c1 - c0) * 128,
                    elem_size=HPAD,
                    single_packet=False)
                # Pt slice for this call span (keeps SBUF down; P is
                # stored per piece so the DRAM row stride stays <64KB)
                Pt = pin.tile([128, GH * 512], bf16, tag="P", name="Pt")
                nc.sync.dma_start(
                    out=Pt[:, :(c1 - c0) * 512],
                    in_=d['P%d' % p][:, c0 * 512:c1 * 512])
                Gmap[(p, si)] = (G, Pt, c0, c1)

            def piece(p):
                qs, spans = spans_of(p)
                Gs = [Gmap[(p, si)] for si in range(len(spans))]
                ch = 0
                for g in range(NGRP):
                    if S_pg[p][g] == 0:
                        continue
                    # two psum tiles, both at partition base 0: half h of
                    # each chunk runs on quadrant (row 64h, col 0) -> DVE
                    # can consume both without a partition move
                    accA = scp.tile([F, 512], f32, space="PSUM", tag="agg",
                                    name="accpA")
                    accB = scp.tile([F, 512], f32, space="PSUM", tag="agg",
                                    name="accpB")
                    for s in range(S_pg[p][g]):
                        gsel = 0
                        while ch >= qs[gsel + 1]:
                            gsel += 1
                        G, Pt, c0, c1 = Gs[gsel]
                        for h in range(2):
                            mi = nc.tensor.matmul(
                                out=(accA if h == 0 else accB)[:, :],
                                lhsT=G[64 * h:64 * h + 64, ch - c0, :F],
                                rhs=Pt[64 * h:64 * h + 64,
                                       (ch - c0) * 512:(ch - c0 + 1) * 512],
                                start=(s == 0), stop=(s == S_pg[p][g] - 1),
                                skip_group_check=True)
                            gate(mi)
                        ch += 1
                    sl = aggS[:, g * 512:(g + 1) * 512]
                    nc.vector.tensor_add(out=sl, in0=sl, in1=accA[:, :])
                    nc.vector.tensor_add(out=sl, in0=sl, in1=accB[:, :])
                    if p == NP - 1:
                        nc.vector.tensor_mul(
                            out=aggb[:, g * 512:(g + 1) * 512],
                            in0=aggS[:, g * 512:(g + 1) * 512],
                            in1=recT_t[:, g * 512:(g + 1) * 512])

            # self-core edges: dense local group adjacency from hnat
            # (runs right at attention end, hidden under the gathers)
            for g in range(NGRP if BIS not in (12, 13) else 0):
                accA = scp.tile([F, 512], f32, space="PSUM", tag="agg",
                                name="accselfA")
                accB = scp.tile([F, 512], f32, space="PSUM", tag="agg",
                                name="accselfB")
                for sb in range(DBLK):
                    for h in range(2):
                        mi = nc.tensor.matmul(
                            out=(accA if h == 0 else accB)[:, :],
                            lhsT=hnat[64 * h:64 * h + 64, sb, :F],
                            rhs=At[64 * h:64 * h + 64,
                                   (g * DBLK + sb) * 512:
                                   (g * DBLK + sb + 1) * 512],
                            start=(sb == 0), stop=(sb == DBLK - 1),
                            skip_group_check=True)
                        gate(mi)
                sl = aggS[:, g * 512:(g + 1) * 512]
                nc.vector.tensor_add(out=sl, in0=sl, in1=accA[:, :])
                nc.vector.tensor_add(out=sl, in0=sl, in1=accB[:, :])
            if BIS in (11, 12, 13):
                nc.vector.memset(aggb[:], 0.0)
            if BIS not in (11, 12, 13):
                for p in range(NP):
                    for si in range(NSPL_P[p]):
                        gather_call(p, si)
                piece(0)
            # hT (bf16) for SAGE lin_r: transpose the 12 h tiles (PE work
            # that fills the gap while gathers run on GpSimd)
            for t in range(DBLK):
                pst = htp.tile([F, 128], bf16, space="PSUM", tag="ht",
                               name="psht")
                ti = nc.tensor.transpose(out=pst[:], in_=hnat[:, t, :F],
                                         identity=ident_t[:])
                if t == 0:
                    add_dep_helper(ti.ins, last_att_mm.ins,
                                   reason="transposes after attention")
                nc.vector.tensor_copy(out=hT[:, t * 128:(t + 1) * 128],
                                      in_=pst[:])
            if BIS not in (11, 12, 13):
                for p in range(1, NP):
                    piece(p)

        # ---------------- SAGE linear + pool + MLP ----------------
        with tc.tile_pool(name="mlpw", bufs=1) as mlpw, \
             tc.tile_pool(name="mlps", bufs=2) as mlps, \
             tc.tile_pool(name="mlpp", bufs=2, space="PSUM") as mlpp:
            WllT_t = mlpw.tile([F, F], bf16, name="WllT_t")
            nc.sync.dma_start(out=WllT_t[:], in_=d['WllT'][:, :])
            WlrT_t = mlpw.tile([F, F], bf16, name="WlrT_t")
            nc.sync.dma_start(out=WlrT_t[:], in_=d['WlrT'][:, :])
            bll_t = mlpw.tile([F, 1], f32, name="bll_t")
            nc.sync.dma_start(out=bll_t[:], in_=d['bll'][:, :])
            Wg1T_t = mlpw.tile([F, 1500], f32, name="Wg1T_t")
            nc.sync.dma_start(out=Wg1T_t[:], in_=d['Wg1T'][:, :])
            bg1_t = mlpw.tile([128, 12], f32, name="bg1_t")
            nc.sync.dma_start(out=bg1_t[:], in_=d['bg1'][:, :])
            Wg2_t = mlpw.tile([128, 12 * 128], f32, name="Wg2_t")
            nc.sync.dma_start(out=Wg2_t[:], in_=d['Wg2Tr'][:, :])
            bg2_t = mlpw.tile([128, 1], f32, name="bg2_t")
            nc.sync.dma_start(out=bg2_t[:], in_=d['bg2'][:, :])
            WoT_t = mlpw.tile([128, 1], f32, name="WoT_t")
            nc.sync.dma_start(out=WoT_t[:], in_=d['WoT'][:, :])

            relu_f = mybir.ActivationFunctionType.Relu
            h2T = mlps.tile([F, IPAD], f32, tag="h2T", name="h2T")
            for cc in range(NGRP):
                ps = mlpp.tile([F, 512], f32, space="PSUM", tag="h2",
                               name="psh2")
                nc.tensor.matmul(out=ps[:], lhsT=WllT_t[:],
                                 rhs=aggb[:, cc * 512:(cc + 1) * 512],
                                 start=True, stop=False,
                                 skip_group_check=True)
                nc.tensor.matmul(out=ps[:], lhsT=WlrT_t[:],
                                 rhs=hT[:, cc * 512:(cc + 1) * 512],
                                 start=False, stop=True,
                                 skip_group_check=True)
                nc.scalar.activation(out=h2T[:, cc * 512:(cc + 1) * 512],
                                     in_=ps[:], func=relu_f, bias=bll_t[:])

            gT = mlps.tile([F, GB], f32, tag="gT", name="gT")
            for g in range(GB):
                lo, hi = GRAPH_BOUNDS[g], GRAPH_BOUNDS[g + 1]
                nc.vector.tensor_reduce(out=gT[:, g:g + 1], in_=h2T[:, lo:hi],
                                        axis=mybir.AxisListType.X,
                                        op=mybir.AluOpType.max)
            g1T = mlps.tile([128, 12, GB], f32, tag="g1T", name="g1T")
            for j in range(12):
                w = min(128, 1500 - j * 128)
                ps = mlpp.tile([128, GB], f32, space="PSUM", tag="g1",
                               name="psg1")
                nc.tensor.matmul(out=ps[:w, :],
                                 lhsT=Wg1T_t[:, j * 128:j * 128 + w],
                                 rhs=gT[:], start=True, stop=True)
                if w < 128:
                    nc.vector.memset(g1T[:, j, :], 0.0)
                nc.scalar.activation(out=g1T[:w, j, :], in_=ps[:w, :],
                                     func=relu_f, bias=bg1_t[:w, j:j + 1])
            g2ps = mlpp.tile([128, GB], f32, space="PSUM", tag="g2",
                             name="g2ps")
            for j in range(12):
                nc.tensor.matmul(out=g2ps[:],
                                 lhsT=Wg2_t[:, j * 128:(j + 1) * 128],
                                 rhs=g1T[:, j, :], start=(j == 0),
                                 stop=(j == 11), skip_group_check=True)
            g2sb = mlps.tile([128, GB], f32, tag="g2sb", name="g2sb")
            nc.vector.tensor_scalar_add(out=g2sb[:], in0=g2ps[:],
                                        scalar1=bg2_t[:])
            ops = mlpp.tile([1, GB], f32, space="PSUM", tag="o", name="ops")
            nc.tensor.matmul(out=ops[:], lhsT=WoT_t[:], rhs=g2sb[:],
                             start=True, stop=True)
            osb = mlps.tile([1, GB], f32, tag="osb", name="osb")
            nc.vector.tensor_scalar_add(out=osb[:], in0=ops[:],
                                        scalar1=float(bo_const))
            nc.sync.dma_start(out=d['out8'][:, :], in_=osb[:])


def _build_program(S_pg, bo_const):
    import concourse.tile as tile
    from concourse import bacc, mybir

    f32 = mybir.dt.float32
    bf16 = mybir.dt.bfloat16
    CH = int(sum(S_pg))
    nc = bacc.Bacc("TRN2", target_bir_lowering=False, debug=False,
                   num_devices=NCORE)

    d = {}

    def dram_in(name, shape, dt=f32):
        d[name] = nc.dram_tensor(name, list(shape), dt, kind="ExternalInput")

    dram_in("xh", (F1, XW), bf16)
    dram_in("xn", (128, JT * F1), bf16)
    dram_in("KQ", (F1, IPAD), bf16)
    dram_in("Wva", (F1, F1), bf16)
    dram_in("Vl", (128, DBLK * F), f32)
    dram_in("ident", (128, 128), bf16)
    dram_in("WllT", (F, F), bf16)
    dram_in("WlrT", (F, F), bf16)
    dram_in("bll", (F, 1))
    dram_in("Wg1T", (F, 1500))
    dram_in("bg1", (128, 12))
    dram_in("Wg2Tr", (128, 12 * 128))
    dram_in("bg2", (128, 1))
    dram_in("WoT", (128, 1))
    dram_in("recipT", (F, IPAD))
    NCH = [int(sum(S_pg[p * NGRP:(p + 1) * NGRP])) for p in range(NP)]
    for p in range(NP):
        dram_in("P%d" % p, (128, NCH[p] * 512), bf16)
    dram_in("Aloc", (128, NGRP * DBLK * 512), bf16)
    d['gidx'] = nc.dram_tensor("gidx", [128, CH * 8], mybir.dt.int16,
                               kind="ExternalInput")
    d['out8'] = nc.dram_tensor("out8", [1, GB], f32, kind="ExternalOutput")
    d['h_loc'] = nc.dram_tensor("h_loc", [ROWS, HPAD], bf16)
    for p in range(NP):
        d['h_full%d' % p] = nc.dram_tensor(
            "h_full%d" % p, [NCORE * PLEN[p], HPAD], bf16,
            addr_space="Shared")

    with tile.TileContext(nc) as tc:
        _emit_body(nc, tc, d, S_pg, bo_const)

    nc.compile()
    return nc


# --------------------------------------------------------------------------
# entry point
# --------------------------------------------------------------------------

_CACHE = {}


def _make_in_maps(inputs):
    x = np.asarray(inputs['x'], np.float32)
    edge_index = np.asarray(inputs['edge_index'])
    w = _prep_weights(inputs)
    xh, xn, KQ = _prep_x(x, w['M'])
    Vl = _prep_vl(x, inputs)
    gidx, Ps, Als, recipT, S_pg = _prep_edges(edge_index)
    ident = np.eye(128, dtype=BF16)
    common = dict(
        xh=xh, xn=xn, Wva=w['Wva'], ident=ident,
        WllT=w['WllT'], WlrT=w['WlrT'],
        bll=w['bll'], Wg1T=w['Wg1T'], bg1=w['bg1'], Wg2Tr=w['Wg2Tr'],
        bg2=w['bg2'], WoT=w['WoT'])
    in_maps = []
    S_pg2 = [list(S_pg[p * NGRP:(p + 1) * NGRP]) for p in range(NP)]
    NCH = [int(sum(S_pg2[p])) for p in range(NP)]
    POFF = [int(sum(NCH[:p])) for p in range(NP)]
    for c in range(NCORE):
        m = dict(common)
        m['KQ'] = KQ[c]
        m['Vl'] = Vl[c]
        m['gidx'] = gidx[c]
        for p in range(NP):
            m['P%d' % p] = np.ascontiguousarray(
                Ps[c][:, POFF[p] * 512:(POFF[p] + NCH[p]) * 512])
        m['Aloc'] = Als[c]
        m['recipT'] = recipT[c]
        in_maps.append(m)
    return in_maps, S_pg, w['bo']


def kernel(**inputs):
    from concourse.bass_utils import run_bass_kernel_spmd

    import os
    in_maps, S_pg, bo = _make_in_maps(inputs)
    key = ('prog', S_pg, bo, os.environ.get('KBISECT', '0'))
    if key not in _CACHE:
        _CACHE[key] = _build_program(S_pg, bo)
    nc = _CACHE[key]

    res = run_bass_kernel_spmd(nc, in_maps, list(range(NCORE)))
    global LAST_RESULT
    LAST_RESULT = res
    out = np.zeros((B, 1), np.float32)
    for c in range(NCORE):
        out[c * GB:(c + 1) * GB, 0] = res.results[c]['out8'].reshape(-1)
    return out


LAST_RESULT = None


# revision 86
# speedup vs baseline: 1.0746x; 1.0539x over previous
"""Trainium2 Bass kernel for nn_GAT_GraphSAGE (N=12000, E=192000, F=35, B=64).

Sharding: attention rows (softmax row i = K_new index) sharded 1500/core on
8 cores; one AllGather of post-attention h per 512-row piece (bf16); SAGE
sharded by dst with a batched dma_gather of h[src] rows + one-hot-matmul
scatter; per-core global-max-pool + MLP head on that core's 8 graphs.

v4 = v2 attention (3 i-pieces of 512, exp on ACT is the in-attention
bottleneck) + restructured SAGE scatter:
- Gather slots deduplicated per (src, piece, 512-col dst group) and packed
  into full 128-slot chunks (24448 -> 16384 slots; GpSimd SWDGE descriptor
  generation measures ~7ns/slot on HW, and is the serial bottleneck after
  attention ends).
- Each chunk scatters via a [64,512] one-hot matmul pair: top/bottom
  64-slot halves write opposite psum partition halves, alternating per
  chunk so all 4 PE quadrants stay busy; the two halves are summed into
  aggS once per (piece, group).
- Self-core edges keep the dense local block adjacency, restructured to
  512-col dst groups with the same quadrant packing.
- MLP head weights in bf16 (kills the fp32 LOW/HIGH double-pass matmuls
  in the tail).
"""
import math
import numpy as np
import ml_dtypes

BF16 = ml_dtypes.bfloat16

N, E, F, B = 12000, 192000, 35, 64
F1 = F + 1
NCORE = 8
ROWS = N // NCORE            # 1500
ICH = 512
NI = 3
IPAD = ICH * NI              # 1536
JT = 94                      # j chunks of 128
JPAD = JT * 128              # 12032
XW = 12064                   # padded x~^T width (covers 7*1500 + 1536)
DBLK = 12                    # dst blocks (128 each) per core
NGRP = 3                     # dst groups of 512 cols
GB = B // NCORE              # 8 graphs per core
HPAD = 128                   # h row padded to 128 bf16 (256B) for dma_gather
GRAPH_BOUNDS = [int(math.ceil(g * (N / B))) for g in range(GB + 1)]
# 6 AllGather pieces of 2 h-blocks each (256 x5, 220): each is triggered
# as soon as its two h blocks are written.  Each AG has a large fixed
# cost (~10-25us, absorbing cross-core skew), so the tail wants exactly
# one final AG, not more.
NP = 6
PLO = [p * 256 for p in range(NP)]
PHI = [min((p + 1) * 256, ROWS) for p in range(NP)]
PLEN = [PHI[p] - PLO[p] for p in range(NP)]
NSPL = 2                     # gather calls per piece


# --------------------------------------------------------------------------
# host-side preprocessing
# --------------------------------------------------------------------------

def _prep_weights(p):
    f64 = np.float64
    f32 = np.float32
    Wq, bq = p['Wq'].astype(f64), p['bq'].astype(f64)
    Wk, bk = p['Wk'].astype(f64), p['bk'].astype(f64)
    Wv, bv = p['Wv'].astype(f64), p['bv'].astype(f64)
    W3c, b3 = p['W3'][:, :, 1].astype(f64), p['b3'].astype(f64)
    W5c, b5 = p['W5'][:, :, 2].astype(f64), p['b5'].astype(f64)
    Wl, bl = p['Wl'].astype(f64), p['bl'].astype(f64)
    Wl1, Wl2, Wl3 = Wl[:, :F], Wl[:, F:2 * F], Wl[:, 2 * F:]

    # K_new = x~ @ Wkn~  (F1 -> F affine, includes 1/sqrt(F))
    Weff = W3c.T @ Wl1.T + W5c.T @ Wl2.T + Wl3.T
    beff = b3 @ Wl1.T + b5 @ Wl2.T + bl
    Wkn = Wk.T @ Weff
    bkn = bk @ Weff + beff
    s = 1.0 / np.sqrt(F)
    Wkn_aug = np.vstack([Wkn, bkn[None, :]]) * s          # [F1, F]
    Wq_aug = np.vstack([Wq.T, bq[None, :]])               # [F1, F]
    M = Wkn_aug @ Wq_aug.T                                # [F1, F1]

    Wva = np.zeros((F1, F1))
    Wva[:F, :F] = Wv.T
    Wva[F, :F] = bv
    Wva[F, F] = 1.0                                       # denominator column
    out = {'M': M, 'Wva': Wva.astype(BF16)}
    out['WllT'] = np.ascontiguousarray(p['Wll'].T).astype(BF16)
    out['WlrT'] = np.ascontiguousarray(p['Wlr'].T).astype(BF16)
    out['bll'] = p['bll'].astype(f32).reshape(F, 1)
    out['Wg1T'] = np.ascontiguousarray(p['Wg1'].T).astype(BF16)  # [35,1500]
    bg1 = np.zeros((128, 12), f32)
    bg1.T.reshape(-1)[:1500] = p['bg1'].astype(f32)
    out['bg1'] = bg1
    w2 = np.zeros((12 * 128, 128), f32)
    w2[:1500, :] = p['Wg2'].T.astype(f32)
    out['Wg2Tr'] = np.ascontiguousarray(
        w2.reshape(12, 128, 128).transpose(1, 0, 2).reshape(128, 12 * 128))
    out['bg2'] = p['bg2'].astype(f32).reshape(128, 1)
    out['WoT'] = p['Wo'].astype(f32).reshape(1, 128).T.copy()
    out['bo'] = float(np.asarray(p['bo']).reshape(-1)[0])
    return out


def _prep_x(x, M):
    """Host: x~^T (bf16), x~ natural chunked (bf16), per-core KQT."""
    x64 = np.asarray(x, np.float64)
    xa = np.concatenate([x64, np.ones((N, 1))], axis=1)       # [N, F1]
    xaT = np.zeros((F1, XW))
    xaT[:, :N] = xa.T                                         # pad cols zero
    xh = xaT.astype(BF16)                                     # [F1, XW]

    # natural chunks for the U accumulation: [128, JT, F1]
    xn = np.zeros((128, JT, F1))
    flat = xaT[:, :JPAD].T                                    # [JPAD, F1]
    xn[:, :, :] = flat.reshape(JT, 128, F1).transpose(1, 0, 2)
    xn = np.ascontiguousarray(xn.reshape(128, JT * F1)).astype(BF16)

    KQ = []
    for c in range(NCORE):
        sl = xaT[:, c * ROWS: c * ROWS + IPAD]                # [F1, IPAD]
        KQ.append(np.ascontiguousarray(M.T @ sl).astype(BF16))
    return xh, xn, KQ


def _prep_vl(x, p):
    """Per-core natural V' local [128, DBLK*F] f32 (for the residual)."""
    f64 = np.float64
    Wv, bv = p['Wv'].astype(f64), p['bv'].astype(f64)
    x64 = np.asarray(x, np.float64)
    V = x64 @ Wv.T + bv                                       # [N, F]
    out = []
    for c in range(NCORE):
        vl = np.zeros((DBLK * 128, F))
        vl[:ROWS] = V[c * ROWS:(c + 1) * ROWS]
        out.append(np.ascontiguousarray(
            vl.reshape(DBLK, 128, F).transpose(1, 0, 2).reshape(128, DBLK * F)
        ).astype(np.float32))
    return out


def _prep_edges(edge_index):
    """Non-self edges keyed (piece p of src, dst group g of 512 cols),
    DEDUPED per (src, p, g): each gathered slot is a unique src row whose
    P columns cover every dst it feeds in that group.  Chunk stream is
    p-major then g-major with S_pg (global max over cores) 128-slot chunks
    per (p, g).  Self-core edges go to a dense local block adjacency Aloc
    laid out per (group, src block).
    """
    src = np.asarray(edge_index[0], np.int64)
    dst = np.asarray(edge_index[1], np.int64)
    deg = np.bincount(dst, minlength=N).astype(np.float64)
    recip = (1.0 / np.maximum(deg, 1.0)).astype(np.float32)

    core_of = dst // ROWS
    dloc = dst - core_of * ROWS
    grp_of = dloc // 512
    sc = src // ROWS
    sr = src - sc * ROWS
    piece_of = np.minimum(sr // 256, NP - 1)
    selfm = sc == core_of
    ns = ~selfm

    # ---- self edges: dense [128, NGRP * DBLK * 512] block adjacency ----
    Als = []
    for c in range(NCORE):
        m = selfm & (core_of == c)
        A = np.zeros((128, NGRP * DBLK * 512), np.float32)
        ssb = sr[m] // 128              # src block
        ssp = sr[m] - ssb * 128         # src pos in block
        g = grp_of[m]
        rel = dloc[m] - g * 512
        np.add.at(A, (ssp, (g * DBLK + ssb) * 512 + rel), 1.0)
        Als.append(np.ascontiguousarray(A.astype(BF16)))

    # ---- non-self: unique (core, piece, grp, src) slots ----
    key = ((core_of * NP + piece_of) * NGRP + grp_of) * N + src
    ukey, inv = np.unique(key[ns], return_inverse=True)
    ucpg = ukey // N
    usrc = ukey - ucpg * N
    uc = ucpg // (NP * NGRP)
    up = (ucpg // NGRP) % NP
    ug = ucpg % NGRP
    counts = np.zeros((NCORE, NP, NGRP), np.int64)
    np.add.at(counts, (uc, up, ug), 1)
    S_pg = np.ceil(counts.max(axis=0) / 128).astype(np.int64)  # [NP, NGRP]
    CH = int(S_pg.sum())

    ch_off = np.zeros((NP, NGRP), np.int64)
    acc = 0
    for p in range(NP):
        for g in range(NGRP):
            ch_off[p, g] = acc
            acc += S_pg[p, g]

    # slot position within its (c,p,g) cell (ukey sorted -> contiguous)
    cell_id = ucpg
    cell_starts = np.searchsorted(cell_id, np.arange(NCORE * NP * NGRP))
    slot_in_cell = np.arange(len(ukey)) - cell_starts[cell_id]
    gslot = ch_off[up, ug] * 128 + slot_in_cell

    # gather position within h_full_p
    plen_arr = np.array(PLEN)[up]
    plo_arr = np.array(PLO)[up]
    su_c = usrc // ROWS
    su_r = usrc - su_c * ROWS
    pos = su_c * plen_arr + (su_r - plo_arr)

    gidx, Ps = [], []
    for c in range(NCORE):
        mu = uc == c
        idx_c = np.zeros(CH * 128, np.int16)
        idx_c[gslot[mu]] = pos[mu].astype(np.int16)
        gidx.append(np.ascontiguousarray(
            np.tile(idx_c.reshape(-1, 16).T, (8, 1))))
        Ps.append(np.zeros((128, CH * 512), np.float32))

    # fill P: each non-self edge scatters its unique slot to its rel col
    e_slot = gslot[inv]
    e_core = core_of[ns]
    e_rel = dloc[ns] - grp_of[ns] * 512
    e_chunk = e_slot // 128
    e_sp = e_slot - e_chunk * 128
    for c in range(NCORE):
        m = e_core == c
        np.add.at(Ps[c], (e_sp[m], e_chunk[m] * 512 + e_rel[m]), 1.0)
    Ps = [np.ascontiguousarray(P.astype(BF16)) for P in Ps]

    recipT = []
    for c in range(NCORE):
        r = np.ones(IPAD, np.float32)
        r[:ROWS] = recip[c * ROWS:(c + 1) * ROWS]
        recipT.append(np.ascontiguousarray(np.broadcast_to(r, (F, IPAD))))
    return gidx, Ps, Als, recipT, tuple(int(v) for v in S_pg.reshape(-1))


# --------------------------------------------------------------------------
# device program
# --------------------------------------------------------------------------

def _emit_body(nc, tc, d, S_pg, bo_const):
    import concourse.tile as tile
    import os
    from concourse import mybir
    from concourse.tile import add_dep_helper

    BIS = int(os.environ.get('KBISECT', '0'))
    f32 = mybir.dt.float32
    bf16 = mybir.dt.bfloat16
    S_pg = [list(S_pg[p * NGRP:(p + 1) * NGRP]) for p in range(NP)]
    NCH = [int(sum(S_pg[p])) for p in range(NP)]       # chunks per piece
    CH = sum(NCH)
    POFF = [int(sum(NCH[:p])) for p in range(NP)]      # piece chunk offsets

    with tc.tile_pool(name="const", bufs=1) as constp, \
         tc.tile_pool(name="main", bufs=1) as main, \
         tc.tile_pool(name="gat", bufs=12) as gat, \
         tc.tile_pool(name="pin", bufs=3) as pin, \
         tc.tile_pool(name="sin", bufs=1) as sin:
        # ---- inputs (leading xhT slices + xn first so exp starts early) ----
        KQT = main.tile([128, IPAD], bf16, name="KQT")
        nc.sync.dma_start(out=KQT[0:F1, :], in_=d['KQ'][:, :])
        nc.sync.dma_start(out=KQT[64:64 + F1, :], in_=d['KQ'][:, :])
        xhT = main.tile([128, XW], bf16, name="xhT")
        xn = main.tile([128, JT * F1], bf16, name="xn")
        HW = XW // 8
        XNW = (JT * F1) // 4
        for q in range(8):
            nc.sync.dma_start(out=xhT[0:F1, q * HW:(q + 1) * HW],
                              in_=d['xh'][:, q * HW:(q + 1) * HW])
            nc.sync.dma_start(out=xhT[64:64 + F1, q * HW:(q + 1) * HW],
                              in_=d['xh'][:, q * HW:(q + 1) * HW])
            if q < 4:
                nc.sync.dma_start(out=xn[:, q * XNW:(q + 1) * XNW],
                                  in_=d['xn'][:, q * XNW:(q + 1) * XNW])
        Wva_t = constp.tile([F1, F1], bf16, name="Wva_t")
        nc.sync.dma_start(out=Wva_t[:], in_=d['Wva'][:, :])
        Vl = main.tile([128, DBLK * F], f32, name="Vl")
        nc.sync.dma_start(out=Vl[:], in_=d['Vl'][:, :])
        ident_t = constp.tile([128, 128], bf16, name="ident_t")
        nc.sync.dma_start(out=ident_t[:], in_=d['ident'][:, :])

        hnat = main.tile([128, DBLK, HPAD], bf16, name="hnat")
        nc.vector.memset(hnat[:, :, F:HPAD], 0.0)
        aggS = main.tile([F, IPAD], f32, name="aggS")
        nc.vector.memset(aggS[:], 0.0)

        # ---------------- attention ----------------
        # groups of 2 j-chunks; one [128,1024] exp per group (2 PSUM banks,
        # double-buffered). U' = sum_j x~_j^T exp[j,:] accumulated in two
        # K-half chains (row groups 0/64 -> UC/UD); V-projection after.
        # UC/UD double-buffered (Up bufs=2) so piece ci+1's U-chain starts
        # without waiting for ci's h post-processing.
        GROUPS = [(g * 2, min(2, JT - g * 2)) for g in range((JT + 1) // 2)]
        exp_f = mybir.ActivationFunctionType.Exp
        with tc.tile_pool(name="mm1p", bufs=2, space="PSUM") as mm1p, \
             tc.tile_pool(name="Up", bufs=2, space="PSUM") as Upp, \
             tc.tile_pool(name="esb", bufs=3) as esb, \
             tc.tile_pool(name="usb", bufs=2) as usb, \
             tc.tile_pool(name="hsm", bufs=4) as hsmall:
            for ci in range(NI):
                UC = Upp.tile([128, ICH], f32, tag="uc", name="UC")
                UD = Upp.tile([128, ICH], f32, tag="ud", name="UD")
                prev = None
                for (j0, glen) in GROUPS:
                    ps = mm1p.tile([128, 2 * ICH], f32, space="PSUM",
                                   tag="s", name="pss")
                    for k in range(glen):
                        j = j0 + k
                        r = 64 * (j & 1)
                        for ch in range(2):
                            nc.tensor.matmul(
                                out=ps[64 * ch:64 * ch + 64,
                                       k * ICH:(k + 1) * ICH],
                                lhsT=xhT[r:r + F1,
                                         j * 128 + 64 * ch:
                                         j * 128 + 64 * ch + 64],
                                rhs=KQT[r:r + F1,
                                        ci * ICH:(ci + 1) * ICH],
                                start=True, stop=True)
                    et = esb.tile([128, 3 * ICH], bf16, tag="e", name="et")
                    nc.scalar.activation(out=et[:, :glen * ICH],
                                         in_=ps[:, :glen * ICH], func=exp_f)
                    if prev is not None:
                        pe, pj0, pglen = prev
                        for k in range(pglen):
                            j = pj0 + k
                            for r in range(2):
                                nc.tensor.matmul(
                                    out=(UC if r == 0 else UD)[0:F1, :],
                                    lhsT=xn[64 * r:64 * r + 64,
                                            j * F1:(j + 1) * F1],
                                    rhs=pe[64 * r:64 * r + 64,
                                           k * ICH:(k + 1) * ICH],
                                    start=(j == 0), stop=False,
                                    skip_group_check=True)
                    prev = (et, j0, glen)
                pe, pj0, pglen = prev
                for k in range(pglen):
                    j = pj0 + k
                    for r in range(2):
                        nc.tensor.matmul(
                            out=(UC if r == 0 else UD)[0:F1, :],
                            lhsT=xn[64 * r:64 * r + 64, j * F1:(j + 1) * F1],
                            rhs=pe[64 * r:64 * r + 64,
                                   k * ICH:(k + 1) * ICH],
                            start=False, stop=(k == pglen - 1),
                            skip_group_check=True)
                # combine K-halves -> U'sb bf16 [F1, 512]
                # (avoid a two-PSUM-operand tensor_tensor: copy then add)
                Ucs = usb.tile([F1, ICH], f32, tag="ucs", name="Ucs")
                nc.vector.tensor_copy(out=Ucs[:], in_=UC[0:F1, :])
                Usb = usb.tile([F1, ICH], bf16, tag="usb", name="Usb")
                nc.vector.tensor_add(out=Usb[:], in0=Ucs[:],
                                     in1=UD[0:F1, :])
                # h natural: hraw[i,g] = sum_f U'sb[f,i] Wva[f,g]
                # (two 64-col halves to stay in the 64x64 tile grid)
                for t in range(4):
                    blk = ci * 4 + t
                    for ch in range(2):
                        last_att_mm = nc.tensor.matmul(
                            out=UD[64 * ch:64 * ch + 64,
                                   t * 128:t * 128 + F1],
                            lhsT=Usb[:, t * 128 + 64 * ch:
                                     t * 128 + 64 * ch + 64],
                            rhs=Wva_t[:],
                            start=True, stop=True, skip_group_check=True)
                    hraw = UD[:, t * 128:t * 128 + F1]
                    rec = hsmall.tile([128, 1], f32, tag="rec", name="rec")
                    nc.vector.reciprocal(out=rec[:], in_=hraw[:, F:F1])
                    hh = hsmall.tile([128, F], f32, tag="hh", name="hh")
                    nc.vector.scalar_tensor_tensor(
                        out=hh[:], in0=hraw[:, :F], scalar=rec[:],
                        in1=Vl[:, blk * F:(blk + 1) * F],
                        op0=mybir.AluOpType.mult,
                        op1=mybir.AluOpType.add)
                    nc.vector.tensor_scalar_max(out=hnat[:, blk, :F],
                                                in0=hh[:], scalar1=0.0)
                    lo = blk * 128
                    nrows = min(128, max(0, ROWS - lo))
                    if nrows > 0:
                        nc.sync.dma_start(
                            out=d['h_loc'][lo:lo + nrows, :],
                            in_=hnat[:nrows, blk, :])
                    # AG a piece as soon as its blocks are all out
                    # (collective outs must be offset-0 full tensors:
                    # sliced outputs silently corrupt on HW)
                    pt = blk // 2 if blk % 2 == 1 else None
                    if pt is not None:
                        nc.gpsimd.collective_compute(
                            "AllGather", mybir.AluOpType.bypass,
                            replica_groups=[list(range(NCORE))],
                            ins=[d['h_loc'][PLO[pt]:PHI[pt], :]],
                            outs=[d['h_full%d' % pt][:, :]])

        # ---------------- SAGE scatter (+ deferred AG piece 2) -----------
        # SBUF pools for G/Pt/idx hoisted to the outer scope (aliasing
        # attention tiles would delay the gathers to attention end).
        aggb = main.tile([F, IPAD], bf16, name="aggb")
        hT = main.tile([F, IPAD], bf16, name="hT")
        idx_t = sin.tile([128, CH * 8], mybir.dt.int16, name="idx_t")
        nc.sync.dma_start(out=idx_t[:], in_=d['gidx'][:, :])
        recT_t = sin.tile([F, IPAD], f32, name="recT_t")
        nc.sync.dma_start(out=recT_t[:], in_=d['recipT'][:, :])
        At = main.tile([128, NGRP * DBLK * 512], bf16, name="At")
        if BIS != 13:
            for g in range(NGRP):
                nc.sync.dma_start(
                    out=At[:, g * DBLK * 512:(g + 1) * DBLK * 512],
                    in_=d['Aloc'][:, g * DBLK * 512:(g + 1) * DBLK * 512])
        with tc.tile_pool(name="scp", bufs=4, space="PSUM") as scp, \
             tc.tile_pool(name="htp", bufs=2, space="PSUM") as htp:
            NSPL_CAP = [NSPL] * NP
            GH = max((NCH[p] + NSPL_CAP[p] - 1) // NSPL_CAP[p]
                     for p in range(NP)) + 1
            # Gate all SAGE PE work behind the last attention matmul (the
            # scheduler's dma_gather cost model is wrong; ungated SAGE PE
            # head-of-line blocks the attention FIFO, and the PSUM banks
            # alias attention pools anyway).
            first_mm = [None]

            def gate(inst):
                if first_mm[0] is None:
                    add_dep_helper(inst.ins, last_att_mm.ins,
                                   reason="SAGE PE after attention")
                    first_mm[0] = inst

            NSPL_P = NSPL_CAP

            def spans_of(p):
                ns = NSPL_P[p]
                qs = [NCH[p] * k // ns for k in range(ns + 1)]
                return qs, [(qs[k], qs[k + 1]) for k in range(ns)]

            Gmap = {}

            def gather_call(p, si):
                qs, spans = spans_of(p)
                c0, c1 = spans[si]
                G = gat.tile([128, GH, HPAD], bf16, tag="G", name="G")
                nc.gpsimd.dma_gather(
                    out_ap=G[:, :c1 - c0, :],
                    in_ap=d['h_full%d' % p][:, :],
                    idxs_ap=idx_t[:, (POFF[p] + c0) * 8:
                                  (POFF[p] + c1) * 8],
                    num_idxs=(c1 - c0) * 128,
                    num_idxs_reg=(c1 - c0) * 128,
                    elem_size=HPAD,
                    single_packet=False)
                # Pt slice for this call span (keeps SBUF down; P is
                # stored per piece so the DRAM row stride stays <64KB)
                Pt = pin.tile([128, GH * 512], bf16, tag="P", name="Pt")
                nc.sync.dma_start(
                    out=Pt[:, :(c1 - c0) * 512],
                    in_=d['P%d' % p][:, c0 * 512:c1 * 512])
                Gmap[(p, si)] = (G, Pt, c0, c1)

            def piece(p):
                qs, spans = spans_of(p)
                Gs = [Gmap[(p, si)] for si in range(len(spans))]
                ch = 0
                for g in range(NGRP):
                    if S_pg[p][g] == 0:
                        continue
                    # two psum tiles, both at partition base 0: half h of
                    # each chunk runs on quadrant (row 64h, col 0) -> DVE
                    # can consume both without a partition move
                    accA = scp.tile([F, 512], f32, space="PSUM", tag="agg",
                                    name="accpA")
                    accB = scp.tile([F, 512], f32, space="PSUM", tag="agg",
                                    name="accpB")
                    for s in range(S_pg[p][g]):
                        gsel = 0
                        while ch >= qs[gsel + 1]:
                            gsel += 1
                        G, Pt, c0, c1 = Gs[gsel]
                        for h in range(2):
                            mi = nc.tensor.matmul(
                                out=(accA if h == 0 else accB)[:, :],
                                lhsT=G[64 * h:64 * h + 64, ch - c0, :F],
                                rhs=Pt[64 * h:64 * h + 64,
                                       (ch - c0) * 512:(ch - c0 + 1) * 512],
                                start=(s == 0), stop=(s == S_pg[p][g] - 1),
                                skip_group_check=True)
                            gate(mi)
                        ch += 1
                    sl = aggS[:, g * 512:(g + 1) * 512]
                    nc.vector.tensor_add(out=sl, in0=sl, in1=accA[:, :])
                    nc.vector.tensor_add(out=sl, in0=sl, in1=accB[:, :])
                    if p == NP - 1:
                        nc.vector.tensor_mul(
                            out=aggb[:, g * 512:(g + 1) * 512],
                            in0=aggS[:, g * 512:(g + 1) * 512],
                            in1=recT_t[:, g * 512:(g + 1) * 512])

            # self-core edges: dense local group adjacency from hnat
            # (runs right at attention end, hidden under the gathers)
            for g in range(NGRP if BIS not in (12, 13) else 0):
                accA = scp.tile([F, 512], f32, space="PSUM", tag="agg",
                                name="accselfA")
                accB = scp.tile([F, 512], f32, space="PSUM", tag="agg",
                                name="accselfB")
                for sb in range(DBLK):
                    for h in range(2):
                        mi = nc.tensor.matmul(
                            out=(accA if h == 0 else accB)[:, :],
                            lhsT=hnat[64 * h:64 * h + 64, sb, :F],
                            rhs=At[64 * h:64 * h + 64,
                                   (g * DBLK + sb) * 512:
                                   (g * DBLK + sb + 1) * 512],
                            start=(sb == 0), stop=(sb == DBLK - 1),
                            skip_group_check=True)
                        gate(mi)
                sl = aggS[:, g * 512:(g + 1) * 512]
                nc.vector.tensor_add(out=sl, in0=sl, in1=accA[:, :])
                nc.vector.tensor_add(out=sl, in0=sl, in1=accB[:, :])
            if BIS in (11, 12, 13):
                nc.vector.memset(aggb[:], 0.0)
            if BIS not in (11, 12, 13):
                for p in range(NP):
                    for si in range(NSPL_P[p]):
                        gather_call(p, si)
                piece(0)
            # hT (bf16) for SAGE lin_r: transpose the 12 h tiles (PE work
            # that fills the gap while gathers run on GpSimd)
            for t in range(DBLK):
                pst = htp.tile([F, 128], bf16, space="PSUM", tag="ht",
                               name="psht")
                ti = nc.tensor.transpose(out=pst[:], in_=hnat[:, t, :F],
                                         identity=ident_t[:])
                if t == 0:
                    add_dep_helper(ti.ins, last_att_mm.ins,
                                   reason="transposes after attention")
                nc.vector.tensor_copy(out=hT[:, t * 128:(t + 1) * 128],
                                      in_=pst[:])
            if BIS not in (11, 12, 13):
                for p in range(1, NP):
                    piece(p)

        # ---------------- SAGE linear + pool + MLP ----------------
        with tc.tile_pool(name="mlpw", bufs=1) as mlpw, \
             tc.tile_pool(name="mlps", bufs=2) as mlps, \
             tc.tile_pool(name="mlpp", bufs=2, space="PSUM") as mlpp:
            WllT_t = mlpw.tile([F, F], bf16, name="WllT_t")
            nc.sync.dma_start(out=WllT_t[:], in_=d['WllT'][:, :])
            WlrT_t = mlpw.tile([F, F], bf16, name="WlrT_t")
            nc.sync.dma_start(out=WlrT_t[:], in_=d['WlrT'][:, :])
            bll_t = mlpw.tile([F, 1], f32, name="bll_t")
            nc.sync.dma_start(out=bll_t[:], in_=d['bll'][:, :])
            Wg1T_t = mlpw.tile([F, 1500], bf16, name="Wg1T_t")
            nc.sync.dma_start(out=Wg1T_t[:], in_=d['Wg1T'][:, :])
            bg1_t = mlpw.tile([128, 12], f32, name="bg1_t")
            nc.sync.dma_start(out=bg1_t[:], in_=d['bg1'][:, :])
            Wg2_t = mlpw.tile([128, 12 * 128], f32, name="Wg2_t")
            nc.sync.dma_start(out=Wg2_t[:], in_=d['Wg2Tr'][:, :])
            bg2_t = mlpw.tile([128, 1], f32, name="bg2_t")
            nc.sync.dma_start(out=bg2_t[:], in_=d['bg2'][:, :])
            WoT_t = mlpw.tile([128, 1], f32, name="WoT_t")
            nc.sync.dma_start(out=WoT_t[:], in_=d['WoT'][:, :])

            relu_f = mybir.ActivationFunctionType.Relu
            h2T = mlps.tile([F, IPAD], f32, tag="h2T", name="h2T")
            for cc in range(NGRP):
                ps = mlpp.tile([F, 512], f32, space="PSUM", tag="h2",
                               name="psh2")
                nc.tensor.matmul(out=ps[:], lhsT=WllT_t[:],
                                 rhs=aggb[:, cc * 512:(cc + 1) * 512],
                                 start=True, stop=False,
                                 skip_group_check=True)
                nc.tensor.matmul(out=ps[:], lhsT=WlrT_t[:],
                                 rhs=hT[:, cc * 512:(cc + 1) * 512],
                                 start=False, stop=True,
                                 skip_group_check=True)
                nc.scalar.activation(out=h2T[:, cc * 512:(cc + 1) * 512],
                                     in_=ps[:], func=relu_f, bias=bll_t[:])

            gT = mlps.tile([F, GB], f32, tag="gT", name="gT")
            for g in range(GB):
                lo, hi = GRAPH_BOUNDS[g], GRAPH_BOUNDS[g + 1]
                nc.vector.tensor_reduce(out=gT[:, g:g + 1], in_=h2T[:, lo:hi],
                                        axis=mybir.AxisListType.X,
                                        op=mybir.AluOpType.max)
            gTb = mlps.tile([F, GB], bf16, tag="gTb", name="gTb")
            nc.vector.tensor_copy(out=gTb[:], in_=gT[:])
            g1T = mlps.tile([128, 12, GB], f32, tag="g1T", name="g1T")
            for j in range(12):
                w = min(128, 1500 - j * 128)
                ps = mlpp.tile([128, GB], f32, space="PSUM", tag="g1",
                               name="psg1")
                nc.tensor.matmul(out=ps[:w, :],
                                 lhsT=Wg1T_t[:, j * 128:j * 128 + w],
                                 rhs=gTb[:], start=True, stop=True)
                if w < 128:
                    nc.vector.memset(g1T[:, j, :], 0.0)
                nc.scalar.activation(out=g1T[:w, j, :], in_=ps[:w, :],
                                     func=relu_f, bias=bg1_t[:w, j:j + 1])
            g2ps = mlpp.tile([128, GB], f32, space="PSUM", tag="g2",
                             name="g2ps")
            for j in range(12):
                nc.tensor.matmul(out=g2ps[:],
                                 lhsT=Wg2_t[:, j * 128:(j + 1) * 128],
                                 rhs=g1T[:, j, :], start=(j == 0),
                                 stop=(j == 11), skip_group_check=True)
            g2sb = mlps.tile([128, GB], f32, tag="g2sb", name="g2sb")
            nc.vector.tensor_scalar_add(out=g2sb[:], in0=g2ps[:],
                                        scalar1=bg2_t[:])
            ops = mlpp.tile([1, GB], f32, space="PSUM", tag="o", name="ops")
            nc.tensor.matmul(out=ops[:], lhsT=WoT_t[:], rhs=g2sb[:],
                             start=True, stop=True)
            osb = mlps.tile([1, GB], f32, tag="osb", name="osb")
            nc.vector.tensor_scalar_add(out=osb[:], in0=ops[:],
                                        scalar1=float(bo_const))
            nc.sync.dma_start(out=d['out8'][:, :], in_=osb[:])


def _build_program(S_pg, bo_const):
    import concourse.tile as tile
    from concourse import bacc, mybir

    f32 = mybir.dt.float32
    bf16 = mybir.dt.bfloat16
    CH = int(sum(S_pg))
    nc = bacc.Bacc("TRN2", target_bir_lowering=False, debug=False,
                   num_devices=NCORE)

    d = {}

    def dram_in(name, shape, dt=f32):
        d[name] = nc.dram_tensor(name, list(shape), dt, kind="ExternalInput")

    dram_in("xh", (F1, XW), bf16)
    dram_in("xn", (128, JT * F1), bf16)
    dram_in("KQ", (F1, IPAD), bf16)
    dram_in("Wva", (F1, F1), bf16)
    dram_in("Vl", (128, DBLK * F), f32)
    dram_in("ident", (128, 128), bf16)
    dram_in("WllT", (F, F), bf16)
    dram_in("WlrT", (F, F), bf16)
    dram_in("bll", (F, 1))
    dram_in("Wg1T", (F, 1500), bf16)
    dram_in("bg1", (128, 12))
    dram_in("Wg2Tr", (128, 12 * 128))
    dram_in("bg2", (128, 1))
    dram_in("WoT", (128, 1))
    dram_in("recipT", (F, IPAD))
    NCH = [int(sum(S_pg[p * NGRP:(p + 1) * NGRP])) for p in range(NP)]
    for p in range(NP):
        dram_in("P%d" % p, (128, NCH[p] * 512), bf16)
    dram_in("Aloc", (128, NGRP * DBLK * 512), bf16)
    d['gidx'] = nc.dram_tensor("gidx", [128, CH * 8], mybir.dt.int16,
                               kind="ExternalInput")
    d['out8'] = nc.dram_tensor("out8", [1, GB], f32, kind="ExternalOutput")
    d['h_loc'] = nc.dram_tensor("h_loc", [ROWS, HPAD], bf16)
    for p in range(NP):
        d['h_full%d' % p] = nc.dram_tensor(
            "h_full%d" % p, [NCORE * PLEN[p], HPAD], bf16,
            addr_space="Shared")

    with tile.TileContext(nc) as tc:
        _emit_body(nc, tc, d, S_pg, bo_const)

    nc.compile()
    return nc


# --------------------------------------------------------------------------
# entry point
# --------------------------------------------------------------------------

_CACHE = {}


def _make_in_maps(inputs):
    x = np.asarray(inputs['x'], np.float32)
    edge_index = np.asarray(inputs['edge_index'])
    w = _prep_weights(inputs)
    xh, xn, KQ = _prep_x(x, w['M'])
    Vl = _prep_vl(x, inputs)
    gidx, Ps, Als, recipT, S_pg = _prep_edges(edge_index)
    ident = np.eye(128, dtype=BF16)
    common = dict(
        xh=xh, xn=xn, Wva=w['Wva'], ident=ident,
        WllT=w['WllT'], WlrT=w['WlrT'],
        bll=w['bll'], Wg1T=w['Wg1T'], bg1=w['bg1'], Wg2Tr=w['Wg2Tr'],
        bg2=w['bg2'], WoT=w['WoT'])
    in_maps = []
    S_pg2 = [list(S_pg[p * NGRP:(p + 1) * NGRP]) for p in range(NP)]
    NCH = [int(sum(S_pg2[p])) for p in range(NP)]
    POFF = [int(sum(NCH[:p])) for p in range(NP)]
    for c in range(NCORE):
        m = dict(common)
        m['KQ'] = KQ[c]
        m['Vl'] = Vl[c]
        m['gidx'] = gidx[c]
        for p in range(NP):
            m['P%d' % p] = np.ascontiguousarray(
                Ps[c][:, POFF[p] * 512:(POFF[p] + NCH[p]) * 512])
        m['Aloc'] = Als[c]
        m['recipT'] = recipT[c]
        in_maps.append(m)
    return in_maps, S_pg, w['bo']


def kernel(**inputs):
    from concourse.bass_utils import run_bass_kernel_spmd

    import os
    in_maps, S_pg, bo = _make_in_maps(inputs)
    key = ('prog', S_pg, bo, os.environ.get('KBISECT', '0'))
    if key not in _CACHE:
        _CACHE[key] = _build_program(S_pg, bo)
    nc = _CACHE[key]

    res = run_bass_kernel_spmd(nc, in_maps, list(range(NCORE)))
    global LAST_RESULT
    LAST_RESULT = res
    out = np.zeros((B, 1), np.float32)
    for c in range(NCORE):
        out[c * GB:(c + 1) * GB, 0] = res.results[c]['out8'].reshape(-1)
    return out


LAST_RESULT = None
